# revision 1
# baseline (speedup 1.0000x reference)
"""Trainium2 Bass kernel for nn_BoxLM_1168231104949 (gnn_message_passing).

Contract: kernel(**inputs) takes the FULL unsharded inputs (as produced by
setup_inputs()) and returns the full output (visit_final_emb,
visit_final_offset), each [50000, 64] float32.

Math notes (validated against the reference in fp64/numpy):
  * lam == 1.0  =>  visit_final_emb == l2norm(center_net(all_center[tail1],
    head1, N_NODES)[:NV]); the graph-2 center_net contributes exactly 0.
  * logits are tiny (|l| < ~1) so the segment softmax is computed with a raw
    exp (no per-segment max subtraction): out = num/den with
    num = seg_sum(exp(l)*emb), den = seg_sum(exp(l)).
  * exp(l) depends only on the tail node, so it is precomputed per node into
    a table T[v] = [exp(l(v))*center(v) | exp(l(v))] (fp16, 128 ch) and the
    edge work reduces to row gathers + segment sums.
  * The five masked/clamped segment maxes for visit_final_offset collapse to
    one masked segment max over (graph1: tail>=NV) + (graph2: all) edges,
    clamped at 0 (the accumulator initialised to 0 provides the clamp, and
    relu commutes with max so raw offsets are gathered).

Distribution: edges are sorted by head on the host and sharded into 8
contiguous head ranges balanced by edge count - each core owns a disjoint
slice of output nodes, no collectives.  Within a core, nodes are ordered by
degree into "slots"; round r gathers the r-th edge of every node with
degree > r via one bulk dma_gather (slot i -> partition i%128, block
i//128 - exactly the accumulator layout).  dma_gather indices are int16, so
rows are fetched in PAIRS (pair idx = tail//2 <= 28671) and the correct
half is selected on-chip with a host-provided parity mask.  Host work is
index bookkeeping (sort/permute/int16 packing) and output re-permutation.
"""

import numpy as np

import concourse.bacc as bacc
import concourse.bass as bass
import concourse.mybir as mybir
import concourse.tile as tile
from concourse.bass_utils import run_bass_kernel_spmd
from concourse.masks import make_identity

F32 = mybir.dt.float32
F16 = mybir.dt.float16
I16 = mybir.dt.int16
I8 = mybir.dt.int8

NV = 50000
NN = 57300
D = 64
NCORES = 8

CHUNK = 512        # table rows per phase-0 chunk
GCOLS = 25         # max 128-slot blocks per gather call

_last_results = {}


# --------------------------------------------------------------------------
# host-side index preprocessing
# --------------------------------------------------------------------------

def _shard_and_rounds(heads, tails, ncores, sent_pair):
    """Sort edges by head, shard into contiguous node ranges balanced by edge
    count, order nodes by degree desc, emit per-round int16 pair-index
    buffers (dma_gather layout) + parity masks.

    Returns (cores, NB, NBLK).  cores[k]: nlo/nhi/order/idx16/mask.
    NB[r] = 128-slot blocks in round r (uniform across cores).
    """
    deg = np.bincount(heads, minlength=NV)
    cum = np.cumsum(deg)
    total = int(cum[-1])
    bounds = [0]
    for k in range(1, ncores):
        bounds.append(int(np.searchsorted(cum, total * k / ncores)))
    bounds.append(NV)

    order_e = np.argsort(heads, kind="stable")
    t_s = tails[order_e]
    node_start = np.zeros(NV + 1, np.int64)
    node_start[1:] = cum

    cores = []
    for k in range(ncores):
        nlo, nhi = bounds[k], bounds[k + 1]
        ldeg = deg[nlo:nhi]
        order = np.argsort(-ldeg, kind="stable")
        cores.append(dict(nlo=nlo, nhi=nhi, order=order,
                          sorted_deg=ldeg[order]))
    R = max(int(c["sorted_deg"][0]) if len(c["sorted_deg"]) else 0
            for c in cores)
    NBLK = max(-(-(c["nhi"] - c["nlo"]) // 128) for c in cores)
    NB = []
    for r in range(R):
        cnt = max(int(np.searchsorted(-c["sorted_deg"], -r, side="left"))
                  for c in cores)
        NB.append(max(1, -(-cnt // 128)))
    CT = sum(NB)
    for c in cores:
        nlo = c["nlo"]
        # per-slot tail (sent = 2*sent_pair for padding), slot-major per round
        pair = np.full((CT * 128,), sent_pair, np.int32)
        par = np.zeros((CT * 128,), np.int8)
        col0 = 0
        for r, nb in enumerate(NB):
            cnt_k = int(np.searchsorted(-c["sorted_deg"], -r, side="left"))
            s = np.arange(cnt_k)
            g = nlo + c["order"][s]
            tr = t_s[node_start[g] + r]
            pair[col0 * 128 + s] = tr >> 1
            par[col0 * 128 + s] = (tr & 1).astype(np.int8)
            col0 += nb
        # int16 dma_gather layout: per round section, slots wrapped into 16
        # partitions ([16, 8*nb], slot i at [i%16, i//16]) replicated x8
        idx16 = np.empty((128, 8 * CT), np.int16)
        col0 = 0
        for r, nb in enumerate(NB):
            vals = pair[col0 * 128:(col0 + nb) * 128]
            sec = vals.reshape(8 * nb, 16).T.astype(np.int16)     # [16, 8nb]
            idx16[:, 8 * col0:8 * (col0 + nb)] = np.tile(sec, (8, 1))
            col0 += nb
        # parity mask [128, CT]: slot j*128+p -> [p, col0+j]
        mask = par.reshape(CT, 128).T.copy()                      # [128, CT]
        c["idx16"] = idx16
        c["mask"] = mask
    return cores, NB, NBLK


# --------------------------------------------------------------------------
# device kernel builder
# --------------------------------------------------------------------------

def _build_nc(cfg):
    TH = cfg["TH"]
    EMB_NB, EMB_NBLK = cfg["EMB_NB"], cfg["EMB_NBLK"]
    OFF_NB, OFF_NBLK = cfg["OFF_NB"], cfg["OFF_NBLK"]
    CE = max(1, sum(EMB_NB))
    CO = max(1, sum(OFF_NB))
    NCH = TH // CHUNK
    gcols = cfg.get("gcols", GCOLS)
    stage_bufs = cfg.get("stage_bufs", 2)

    nc = bacc.Bacc(None, target_bir_lowering=False, debug=False,
                   num_devices=NCORES, num_swdge_queues=2)

    centerT = nc.dram_tensor("center_t", [D, TH], F32, kind="ExternalInput")
    offcat = nc.dram_tensor("offcat", [TH, D], F32, kind="ExternalInput")
    w1t = nc.dram_tensor("w1t", [D, D], F32, kind="ExternalInput")
    w2t = nc.dram_tensor("w2t", [D, D], F32, kind="ExternalInput")
    b1 = nc.dram_tensor("b1", [D, 1], F32, kind="ExternalInput")
    b2 = nc.dram_tensor("b2", [D, 1], F32, kind="ExternalInput")
    idx_e = nc.dram_tensor("idx_e", [128, 8 * CE], I16, kind="ExternalInput")
    idx_o = nc.dram_tensor("idx_o", [128, 8 * CO], I16, kind="ExternalInput")
    mask_e = nc.dram_tensor("mask_e", [128, CE], I8, kind="ExternalInput")
    mask_o = nc.dram_tensor("mask_o", [128, CO], I8, kind="ExternalInput")

    tp = nc.dram_tensor("tp", [TH, 2 * D], F16)   # internal node table

    emb_out = nc.dram_tensor("emb_out", [128, EMB_NBLK * D], F32,
                             kind="ExternalOutput")
    off_out = nc.dram_tensor("off_out", [128, OFF_NBLK * D], F32,
                             kind="ExternalOutput")

    tp_pair = tp[:].rearrange("(u two) c -> u (two c)", two=2)       # [TH/2, 256]
    off_pair = offcat[:].rearrange("(u two) c -> u (two c)", two=2)  # [TH/2, 128]

    with tile.TileContext(nc) as tc:
        with (
            tc.tile_pool(name="persist", bufs=1) as pp,
            tc.tile_pool(name="ph0", bufs=3) as p0,
            tc.tile_pool(name="ph0psum", bufs=2, space="PSUM") as pps,
            tc.tile_pool(name="stage", bufs=stage_bufs) as ps,
            tc.tile_pool(name="selp", bufs=2) as psel,
        ):
            # ---- constants -------------------------------------------------
            w1t_sb = pp.tile([D, D], F32, tag="w1t")
            w2t_sb = pp.tile([D, D], F32, tag="w2t")
            b1_sb = pp.tile([D, 1], F32, tag="b1")
            b2_sb = pp.tile([D, 1], F32, tag="b2")
            ident = pp.tile([128, 128], F32, tag="ident")
            zrow = pp.tile([2, 2 * D], F16, tag="zrow")
            nc.sync.dma_start(out=w1t_sb[:], in_=w1t[:])
            nc.sync.dma_start(out=w2t_sb[:], in_=w2t[:])
            nc.sync.dma_start(out=b1_sb[:], in_=b1[:])
            nc.sync.dma_start(out=b2_sb[:], in_=b2[:])
            make_identity(nc, ident[:])
            nc.vector.memset(zrow[:], 0.0)

            # ---- persistent phase-1 state ---------------------------------
            idx_e_sb = pp.tile([128, 8 * CE], I16, tag="idx_e")
            idx_o_sb = pp.tile([128, 8 * CO], I16, tag="idx_o")
            mask_e_sb = pp.tile([128, CE], I8, tag="mask_e")
            mask_o_sb = pp.tile([128, CO], I8, tag="mask_o")
            acc_e = pp.tile([128, EMB_NBLK * 128], F32, tag="acc_e")
            acc_o = pp.tile([128, OFF_NBLK * D], F32, tag="acc_o")
            nc.sync.dma_start(out=idx_e_sb[:], in_=idx_e[:])
            nc.sync.dma_start(out=idx_o_sb[:], in_=idx_o[:])
            nc.sync.dma_start(out=mask_e_sb[:], in_=mask_e[:])
            nc.sync.dma_start(out=mask_o_sb[:], in_=mask_o[:])
            nc.vector.memset(acc_e[:], 0.0)
            nc.vector.memset(acc_o[:], 0.0)

            # ---- offset path: pair-gather raw offsets, select, max --------
            # (emitted first: needs no table, overlaps the table build)
            col0 = 0
            for r, nb in enumerate(OFF_NB):
                for j0 in range(0, nb, gcols):
                    w = min(gcols, nb - j0)
                    cl, cr = col0 + j0, col0 + j0 + w
                    st = ps.tile([128, gcols * 2 * D], F32, tag="stag_o")
                    st3 = st[:, :w * 2 * D].rearrange(
                        "p (j c) -> p j c", c=2 * D)
                    nc.gpsimd.dma_gather(
                        out_ap=st3, in_ap=off_pair,
                        idxs_ap=idx_o_sb[:, 8 * cl:8 * cr],
                        num_idxs=128 * w, num_idxs_reg=128 * w,
                        elem_size=2 * D, single_packet=False, queue_num=1)
                    sel = psel.tile([128, gcols * D], F32, tag="sel_o")
                    sv = sel[:, :w * D]
                    nc.scalar.copy(out=sv, in_=st3[:, :, 0:D])
                    nc.vector.copy_predicated(
                        out=sv.rearrange("p (j c) -> p j c", c=D),
                        mask=mask_o_sb[:, cl:cr].to_broadcast([128, w, D]),
                        data=st3[:, :, D:2 * D])
                    nc.vector.tensor_tensor(
                        out=acc_o[:, j0 * D:(j0 + w) * D],
                        in0=acc_o[:, j0 * D:(j0 + w) * D],
                        in1=sv, op=mybir.AluOpType.max)
                col0 += nb

            # ---- phase 0: node table  tp[v] = [exp(l)*c | exp(l)] fp16 ----
            for ch in range(NCH):
                sl = slice(ch * CHUNK, (ch + 1) * CHUNK)
                ct = p0.tile([D, CHUNK], F32, tag="ct")
                nc.sync.dma_start(out=ct[:], in_=centerT[:, sl])
                ph = pps.tile([D, CHUNK], F32, tag="ph")
                nc.tensor.matmul(out=ph[:], lhsT=w1t_sb[:], rhs=ct[:],
                                 start=True, stop=True)
                hT = p0.tile([D, CHUNK], F32, tag="hT")
                nc.scalar.activation(out=hT[:], in_=ph[:],
                                     func=mybir.ActivationFunctionType.Relu,
                                     bias=b1_sb[:])
                pl = pps.tile([D, CHUNK], F32, tag="pl")
                nc.tensor.matmul(out=pl[:], lhsT=w2t_sb[:], rhs=hT[:],
                                 start=True, stop=True)
                eT = p0.tile([D, CHUNK], F32, tag="eT")
                nc.scalar.activation(out=eT[:], in_=pl[:],
                                     func=mybir.ActivationFunctionType.Exp,
                                     bias=b2_sb[:])
                pT = p0.tile([D, CHUNK], F32, tag="pT")
                nc.vector.tensor_tensor(out=pT[:], in0=eT[:], in1=ct[:],
                                        op=mybir.AluOpType.mult)
                pt = pps.tile([128, CHUNK], F32, tag="pt")
                for q in range(CHUNK // 128):
                    nc.tensor.transpose(out=pt[:, q * 128:q * 128 + D],
                                        in_=pT[:, q * 128:(q + 1) * 128],
                                        identity=ident[:D, :D])
                    nc.tensor.transpose(out=pt[:, q * 128 + D:(q + 1) * 128],
                                        in_=eT[:, q * 128:(q + 1) * 128],
                                        identity=ident[:D, :D])
                ot = p0.tile([128, CHUNK], F16, tag="ot")
                half = CHUNK // 2
                nc.vector.tensor_copy(out=ot[:, :half], in_=pt[:, :half])
                nc.scalar.copy(out=ot[:, half:], in_=pt[:, half:])
                nc.sync.dma_start(
                    out=tp[sl, :].rearrange("(q p) c -> p q c", p=128),
                    in_=ot[:].rearrange("p (q c) -> p q c", c=128),
                )
            # zero the sentinel pair (last two rows)
            nc.sync.dma_start(out=tp[TH - 2:TH, :], in_=zrow[:])

            # ---- phase 1: emb pair-gathers, select, add -------------------
            col0 = 0
            for r, nb in enumerate(EMB_NB):
                for j0 in range(0, nb, gcols):
                    w = min(gcols, nb - j0)
                    cl, cr = col0 + j0, col0 + j0 + w
                    st = ps.tile([128, gcols * 4 * D], F16, tag="stag_e")
                    st3 = st[:, :w * 4 * D].rearrange(
                        "p (j c) -> p j c", c=4 * D)
                    nc.gpsimd.dma_gather(
                        out_ap=st3, in_ap=tp_pair,
                        idxs_ap=idx_e_sb[:, 8 * cl:8 * cr],
                        num_idxs=128 * w, num_idxs_reg=128 * w,
                        elem_size=4 * D, single_packet=False, queue_num=0)
                    sel = psel.tile([128, gcols * 2 * D], F16, tag="sel_e")
                    sv = sel[:, :w * 2 * D]
                    nc.scalar.copy(out=sv, in_=st3[:, :, 0:2 * D])
                    nc.vector.copy_predicated(
                        out=sv.rearrange("p (j c) -> p j c", c=2 * D),
                        mask=mask_e_sb[:, cl:cr].to_broadcast([128, w, 2 * D]),
                        data=st3[:, :, 2 * D:4 * D])
                    nc.vector.tensor_add(
                        out=acc_e[:, j0 * 128:(j0 + w) * 128],
                        in0=acc_e[:, j0 * 128:(j0 + w) * 128],
                        in1=sv)
                col0 += nb

            # ---- finals: v = num/den, l2norm, write out -------------------
            acc3 = acc_e[:].rearrange("p (b c) -> p b c", c=128)
            num = acc3[:, :, 0:D]
            den = acc3[:, :, D:2 * D]
            nc.vector.tensor_scalar_max(den, den, 1e-30)
            nc.vector.reciprocal(den, den)
            v = pp.tile([128, EMB_NBLK * D], F32, tag="vfin")
            v3 = v[:].rearrange("p (b c) -> p b c", c=D)
            nc.vector.tensor_tensor(out=v3, in0=num, in1=den,
                                    op=mybir.AluOpType.mult)
            ssq = pp.tile([128, EMB_NBLK], F32, tag="ssq")
            for b in range(EMB_NBLK):
                sqs = p0.tile([128, D], F32, tag="sqscratch")
                nc.scalar.activation(
                    out=sqs[:], in_=v[:, b * D:(b + 1) * D],
                    func=mybir.ActivationFunctionType.Square,
                    accum_out=ssq[:, b:b + 1])
            nc.vector.tensor_scalar_max(ssq[:], ssq[:], 1e-24)
            nc.scalar.sqrt(out=ssq[:], in_=ssq[:])
            nc.vector.reciprocal(ssq[:], ssq[:])
            for b in range(EMB_NBLK):
                nc.scalar.mul(out=v[:, b * D:(b + 1) * D],
                              in_=v[:, b * D:(b + 1) * D],
                              mul=ssq[:, b:b + 1])
            nc.sync.dma_start(out=emb_out[:], in_=v[:])
            nc.sync.dma_start(out=off_out[:], in_=acc_o[:])

    nc.compile()
    return nc


# --------------------------------------------------------------------------
# top-level entry
# --------------------------------------------------------------------------

def _prepare(inputs, TH):
    sent_pair = (TH - 2) // 2
    h1 = np.asarray(inputs["head1"])
    t1 = np.asarray(inputs["tail1"])
    h2 = np.asarray(inputs["head2"])
    t2 = np.asarray(inputs["tail2"])

    m = h1 < NV
    emb_cores, EMB_NB, EMB_NBLK = _shard_and_rounds(
        h1[m], t1[m], NCORES, sent_pair)

    m1 = (h1 < NV) & (t1 >= NV)
    m2 = h2 < NV
    ho = np.concatenate([h1[m1], h2[m2]])
    to = np.concatenate([t1[m1], t2[m2]])
    off_cores, OFF_NB, OFF_NBLK = _shard_and_rounds(ho, to, NCORES, sent_pair)

    all_center = np.concatenate(
        [inputs["visit_center"], inputs["ccs_center"], inputs["icd_center"]], 0)
    all_offset = np.concatenate(
        [inputs["visit_offset"], inputs["ccs_offset"], inputs["icd_offset"]], 0)
    center_pad = np.zeros((TH, D), np.float32)
    center_pad[:len(all_center)] = all_center
    offset_pad = np.zeros((TH, D), np.float32)
    offset_pad[:len(all_offset)] = all_offset
    return dict(emb_cores=emb_cores, EMB_NB=EMB_NB, EMB_NBLK=EMB_NBLK,
                off_cores=off_cores, OFF_NB=OFF_NB, OFF_NBLK=OFF_NBLK,
                center_t=np.ascontiguousarray(center_pad.T),
                offcat=offset_pad)


def kernel(**inputs):
    TH = -(-NN // CHUNK) * CHUNK          # 57344
    prep = _prepare(inputs, TH)

    cfg = dict(TH=TH,
               EMB_NB=list(prep["EMB_NB"]), EMB_NBLK=prep["EMB_NBLK"],
               OFF_NB=list(prep["OFF_NB"]), OFF_NBLK=prep["OFF_NBLK"],
               gcols=12, stage_bufs=5)
    nc = _build_nc(cfg)

    common = dict(
        center_t=prep["center_t"],
        offcat=prep["offcat"],
        w1t=np.ascontiguousarray(np.asarray(inputs["att_w1"]).T),
        w2t=np.ascontiguousarray(np.asarray(inputs["att_w2"]).T),
        b1=np.asarray(inputs["att_b1"]).reshape(D, 1),
        b2=np.asarray(inputs["att_b2"]).reshape(D, 1),
    )
    in_maps = []
    for k in range(NCORES):
        m = dict(common)
        m["idx_e"] = prep["emb_cores"][k]["idx16"]
        m["idx_o"] = prep["off_cores"][k]["idx16"]
        m["mask_e"] = prep["emb_cores"][k]["mask"]
        m["mask_o"] = prep["off_cores"][k]["mask"]
        in_maps.append(m)

    res = run_bass_kernel_spmd(nc, in_maps, core_ids=list(range(NCORES)))
    _last_results["res"] = res
    _last_results["nc"] = nc
    _last_results["in_maps"] = in_maps

    emb = np.zeros((NV, D), np.float32)
    off = np.zeros((NV, D), np.float32)
    for k in range(NCORES):
        ce = prep["emb_cores"][k]
        co = prep["off_cores"][k]
        eo = res.results[k]["emb_out"].reshape(128, prep["EMB_NBLK"], D)
        oo = res.results[k]["off_out"].reshape(128, prep["OFF_NBLK"], D)
        eo = eo.transpose(1, 0, 2).reshape(-1, D)
        oo = oo.transpose(1, 0, 2).reshape(-1, D)
        emb[ce["nlo"] + ce["order"]] = eo[:ce["nhi"] - ce["nlo"]]
        off[co["nlo"] + co["order"]] = oo[:co["nhi"] - co["nlo"]]
    return emb, off



# revision 2
# speedup vs baseline: 5.3358x; 5.3358x over previous
"""Trainium2 Bass kernel for nn_BoxLM_1168231104949 (gnn_message_passing).

Contract: kernel(**inputs) takes the FULL unsharded inputs (as produced by
setup_inputs()) and returns the full output (visit_final_emb,
visit_final_offset), each [50000, 64] float32.

Math notes (validated against the reference in fp64/numpy):
  * lam == 1.0  =>  visit_final_emb == l2norm(center_net(all_center[tail1],
    head1, N_NODES)[:NV]); the graph-2 center_net contributes exactly 0.
  * logits are tiny (|l| < ~1) so the segment softmax is computed with a raw
    exp (no per-segment max subtraction): out = num/den with
    num = seg_sum(exp(l)*emb), den = seg_sum(exp(l)).
  * exp(l) depends only on the tail node, so it is precomputed per node into
    a table T[v] = [exp(l(v))*center(v) | exp(l(v))] (fp16, 128 ch) and the
    edge work reduces to row gathers + segment sums.
  * The five masked/clamped segment maxes for visit_final_offset collapse to
    one masked segment max over (graph1: tail>=NV) + (graph2: all) edges,
    clamped at 0 (the accumulator initialised to 0 provides the clamp, and
    relu commutes with max so raw offsets are gathered).

Distribution: edges are sorted by head on the host and sharded into 8
contiguous head ranges balanced by edge count - each core owns a disjoint
slice of output nodes.  Within a core, nodes are ordered by degree into
"slots"; round r gathers the r-th edge of every node with degree > r via one
bulk dma_gather (slot i -> partition i%128, block i//128 - exactly the
accumulator layout).  dma_gather indices are int16, so rows are fetched in
PAIRS (pair idx = tail//2 <= 28671) and the correct half is selected
on-chip with a host-provided parity mask.

Wall time through the axon tunnel is dominated by host<->device transfer
(~40 MB/s measured), so inputs are shipped minimally: the node tables are
SHARDED 1/8 per core in fp16 and reconstructed on-device with NeuronLink
AllGather (each core builds its slice of the exp table from its center
shard before the gather); dma_gather index buffers are shipped un-replicated
[16, 8*CT] and expanded to the required [128, 8*CT] layout on-device; the
outputs return as fp16.
"""

import numpy as np

import concourse.bacc as bacc
import concourse.bass as bass
import concourse.mybir as mybir
import concourse.tile as tile
from concourse.bass_utils import run_bass_kernel_spmd
from concourse.masks import make_identity

F32 = mybir.dt.float32
F16 = mybir.dt.float16
I16 = mybir.dt.int16
I8 = mybir.dt.int8

NV = 50000
NN = 57300
D = 64
NCORES = 8

CHUNK = 512        # table rows per phase-0 chunk
GCOLS = 25         # max 128-slot blocks per gather call

_last_results = {}


# --------------------------------------------------------------------------
# host-side index preprocessing
# --------------------------------------------------------------------------

def _shard_and_rounds(heads, tails, ncores, sent_pair):
    """Sort edges by head, shard into contiguous node ranges balanced by edge
    count, order nodes by degree desc, emit per-round int16 pair-index
    buffers (un-replicated dma_gather layout) + parity masks.

    Returns (cores, NB, NBLK).  cores[k]: nlo/nhi/order/idx16/mask.
    NB[r] = 128-slot blocks in round r (uniform across cores).
    """
    deg = np.bincount(heads, minlength=NV)
    cum = np.cumsum(deg)
    total = int(cum[-1])
    bounds = [0]
    for k in range(1, ncores):
        bounds.append(int(np.searchsorted(cum, total * k / ncores)))
    bounds.append(NV)

    order_e = np.argsort(heads, kind="stable")
    t_s = tails[order_e]
    node_start = np.zeros(NV + 1, np.int64)
    node_start[1:] = cum

    cores = []
    for k in range(ncores):
        nlo, nhi = bounds[k], bounds[k + 1]
        ldeg = deg[nlo:nhi]
        order = np.argsort(-ldeg, kind="stable")
        cores.append(dict(nlo=nlo, nhi=nhi, order=order,
                          sorted_deg=ldeg[order]))
    R = max(int(c["sorted_deg"][0]) if len(c["sorted_deg"]) else 0
            for c in cores)
    NBLK = max(-(-(c["nhi"] - c["nlo"]) // 128) for c in cores)
    NB = []
    for r in range(R):
        cnt = max(int(np.searchsorted(-c["sorted_deg"], -r, side="left"))
                  for c in cores)
        NB.append(max(1, -(-cnt // 128)))
    CT = sum(NB)
    for c in cores:
        nlo = c["nlo"]
        # per-slot tail (sent = 2*sent_pair for padding), slot-major per round
        pair = np.full((CT * 128,), sent_pair, np.int32)
        par = np.zeros((CT * 128,), np.int8)
        col0 = 0
        for r, nb in enumerate(NB):
            cnt_k = int(np.searchsorted(-c["sorted_deg"], -r, side="left"))
            s = np.arange(cnt_k)
            g = nlo + c["order"][s]
            tr = t_s[node_start[g] + r]
            pair[col0 * 128 + s] = tr >> 1
            par[col0 * 128 + s] = (tr & 1).astype(np.int8)
            col0 += nb
        # int16 dma_gather layout: per round section, slots wrapped into 16
        # partitions ([16, 8*nb], slot i at [i%16, i//16]); the x8 partition
        # replication the gather engine wants is done on-device.
        idx16 = np.empty((16, 8 * CT), np.int16)
        col0 = 0
        for r, nb in enumerate(NB):
            vals = pair[col0 * 128:(col0 + nb) * 128]
            sec = vals.reshape(8 * nb, 16).T.astype(np.int16)     # [16, 8nb]
            idx16[:, 8 * col0:8 * (col0 + nb)] = sec
            col0 += nb
        # parity mask [128, CT]: slot j*128+p -> [p, col0+j]
        mask = par.reshape(CT, 128).T.copy()                      # [128, CT]
        c["idx16"] = idx16
        c["mask"] = mask
    return cores, NB, NBLK


# --------------------------------------------------------------------------
# device kernel builder
# --------------------------------------------------------------------------

def _build_nc(cfg):
    TH = cfg["TH"]
    SH = TH // NCORES
    EMB_NB, EMB_NBLK = cfg["EMB_NB"], cfg["EMB_NBLK"]
    OFF_NB, OFF_NBLK = cfg["OFF_NB"], cfg["OFF_NBLK"]
    CE = max(1, sum(EMB_NB))
    CO = max(1, sum(OFF_NB))
    NCH = SH // CHUNK
    gcols = cfg.get("gcols", GCOLS)
    stage_bufs = cfg.get("stage_bufs", 2)
    GROUP = [list(range(NCORES))]

    nc = bacc.Bacc(None, target_bir_lowering=False, debug=False,
                   num_devices=NCORES, num_swdge_queues=2)

    center_sh = nc.dram_tensor("center_sh", [SH, D], F16, kind="ExternalInput")
    offset_sh = nc.dram_tensor("offset_sh", [SH, D], F16, kind="ExternalInput")
    w1t = nc.dram_tensor("w1t", [D, D], F32, kind="ExternalInput")
    w2t = nc.dram_tensor("w2t", [D, D], F32, kind="ExternalInput")
    b1 = nc.dram_tensor("b1", [D, 1], F32, kind="ExternalInput")
    b2 = nc.dram_tensor("b2", [D, 1], F32, kind="ExternalInput")
    idx_e = nc.dram_tensor("idx_e", [16, 8 * CE], I16, kind="ExternalInput")
    idx_o = nc.dram_tensor("idx_o", [16, 8 * CO], I16, kind="ExternalInput")
    mask_e = nc.dram_tensor("mask_e", [128, CE], I8, kind="ExternalInput")
    mask_o = nc.dram_tensor("mask_o", [128, CO], I8, kind="ExternalInput")

    tp_b = nc.dram_tensor("tp_b", [SH, 2 * D], F16)     # local table shard
    tp = nc.dram_tensor("tp", [TH, 2 * D], F16)         # AllGather output
    off_b = nc.dram_tensor("off_b", [SH, D], F16)
    offf = nc.dram_tensor("offf", [TH, D], F16)

    emb_out = nc.dram_tensor("emb_out", [128, EMB_NBLK * D], F16,
                             kind="ExternalOutput")
    off_out = nc.dram_tensor("off_out", [128, OFF_NBLK * D], F16,
                             kind="ExternalOutput")

    tp_pair = tp[:].rearrange("(u two) c -> u (two c)", two=2)     # [TH/2, 4D]
    off_pair = offf[:].rearrange("(u two) c -> u (two c)", two=2)  # [TH/2, 2D]

    with tile.TileContext(nc) as tc:
        with (
            tc.tile_pool(name="persist", bufs=1) as pp,
            tc.tile_pool(name="ph0", bufs=3) as p0,
            tc.tile_pool(name="ph0psum", bufs=2, space="PSUM") as pps,
            tc.tile_pool(name="stage", bufs=stage_bufs) as ps,
            tc.tile_pool(name="selp", bufs=2) as psel,
        ):
            # ---- offset table: bounce shard -> AllGather (early) -----------
            nc.sync.dma_start(out=off_b[:], in_=offset_sh[:])
            nc.gpsimd.collective_compute(
                "AllGather", mybir.AluOpType.bypass, replica_groups=GROUP,
                ins=[off_b.ap().opt()], outs=[offf.ap().opt()])

            # ---- constants -------------------------------------------------
            w1t_sb = pp.tile([D, D], F32, tag="w1t")
            w2t_sb = pp.tile([D, D], F32, tag="w2t")
            b1_sb = pp.tile([D, 1], F32, tag="b1")
            b2_sb = pp.tile([D, 1], F32, tag="b2")
            ident = pp.tile([128, 128], F32, tag="ident")
            zrow = pp.tile([2, 2 * D], F16, tag="zrow")
            nc.sync.dma_start(out=w1t_sb[:], in_=w1t[:])
            nc.sync.dma_start(out=w2t_sb[:], in_=w2t[:])
            nc.sync.dma_start(out=b1_sb[:], in_=b1[:])
            nc.sync.dma_start(out=b2_sb[:], in_=b2[:])
            make_identity(nc, ident[:])
            nc.vector.memset(zrow[:], 0.0)

            # ---- persistent phase-1 state ---------------------------------
            # idx buffers are shipped [16, 8*C] and replicated to the
            # [128, 8*C] layout dma_gather wants (8 copies along partitions).
            idx_e_sb = pp.tile([128, 8 * CE], I16, tag="idx_e")
            idx_o_sb = pp.tile([128, 8 * CO], I16, tag="idx_o")
            mask_e_sb = pp.tile([128, CE], I8, tag="mask_e")
            mask_o_sb = pp.tile([128, CO], I8, tag="mask_o")
            acc_e = pp.tile([128, EMB_NBLK * 128], F32, tag="acc_e")
            acc_o = pp.tile([128, OFF_NBLK * D], F32, tag="acc_o")
            for r in range(8):
                nc.sync.dma_start(out=idx_e_sb[16 * r:16 * (r + 1), :],
                                  in_=idx_e[:])
                nc.sync.dma_start(out=idx_o_sb[16 * r:16 * (r + 1), :],
                                  in_=idx_o[:])
            nc.sync.dma_start(out=mask_e_sb[:], in_=mask_e[:])
            nc.sync.dma_start(out=mask_o_sb[:], in_=mask_o[:])
            nc.vector.memset(acc_e[:], 0.0)
            nc.vector.memset(acc_o[:], 0.0)

            # ---- offset path: pair-gather fp16 offsets, select, max -------
            # (emitted first: needs only the early AllGather, overlaps the
            # table build)
            col0 = 0
            for r, nb in enumerate(OFF_NB):
                for j0 in range(0, nb, gcols):
                    w = min(gcols, nb - j0)
                    cl, cr = col0 + j0, col0 + j0 + w
                    st = ps.tile([128, gcols * 2 * D], F16, tag="stag_o")
                    st3 = st[:, :w * 2 * D].rearrange(
                        "p (j c) -> p j c", c=2 * D)
                    nc.gpsimd.dma_gather(
                        out_ap=st3, in_ap=off_pair,
                        idxs_ap=idx_o_sb[:, 8 * cl:8 * cr],
                        num_idxs=128 * w, num_idxs_reg=128 * w,
                        elem_size=2 * D, single_packet=False, queue_num=1)
                    sel = psel.tile([128, gcols * D], F16, tag="sel_o")
                    sv = sel[:, :w * D]
                    nc.scalar.copy(out=sv, in_=st3[:, :, 0:D])
                    nc.vector.copy_predicated(
                        out=sv.rearrange("p (j c) -> p j c", c=D),
                        mask=mask_o_sb[:, cl:cr].to_broadcast([128, w, D]),
                        data=st3[:, :, D:2 * D])
                    nc.vector.tensor_tensor(
                        out=acc_o[:, j0 * D:(j0 + w) * D],
                        in0=acc_o[:, j0 * D:(j0 + w) * D],
                        in1=sv, op=mybir.AluOpType.max)
                col0 += nb

            # ---- phase 0: local shard of node table  tp[v] = [e*c | e] ----
            for ch in range(NCH):
                sl = slice(ch * CHUNK, (ch + 1) * CHUNK)
                ld16 = p0.tile([128, (CHUNK // 128) * D], F16, tag="ld16")
                nc.sync.dma_start(
                    out=ld16[:].rearrange("p (q d) -> p q d", d=D),
                    in_=center_sh[sl, :].rearrange("(q p) d -> p q d", p=128))
                ld32 = p0.tile([128, (CHUNK // 128) * D], F32, tag="ld32")
                nc.vector.tensor_copy(out=ld32[:], in_=ld16[:])
                ctp = pps.tile([D, CHUNK], F32, tag="ctp")
                for q in range(CHUNK // 128):
                    nc.tensor.transpose(out=ctp[:, q * 128:(q + 1) * 128],
                                        in_=ld32[:, q * D:(q + 1) * D],
                                        identity=ident[:])
                ct = p0.tile([D, CHUNK], F32, tag="ct")
                nc.scalar.copy(out=ct[:], in_=ctp[:])
                ph = pps.tile([D, CHUNK], F32, tag="ph")
                nc.tensor.matmul(out=ph[:], lhsT=w1t_sb[:], rhs=ct[:],
                                 start=True, stop=True)
                hT = p0.tile([D, CHUNK], F32, tag="hT")
                nc.scalar.activation(out=hT[:], in_=ph[:],
                                     func=mybir.ActivationFunctionType.Relu,
                                     bias=b1_sb[:])
                pl = pps.tile([D, CHUNK], F32, tag="pl")
                nc.tensor.matmul(out=pl[:], lhsT=w2t_sb[:], rhs=hT[:],
                                 start=True, stop=True)
                eT = p0.tile([D, CHUNK], F32, tag="eT")
                nc.scalar.activation(out=eT[:], in_=pl[:],
                                     func=mybir.ActivationFunctionType.Exp,
                                     bias=b2_sb[:])
                pT = p0.tile([D, CHUNK], F32, tag="pT")
                nc.vector.tensor_tensor(out=pT[:], in0=eT[:], in1=ct[:],
                                        op=mybir.AluOpType.mult)
                pt = pps.tile([128, CHUNK], F32, tag="pt")
                for q in range(CHUNK // 128):
                    nc.tensor.transpose(out=pt[:, q * 128:q * 128 + D],
                                        in_=pT[:, q * 128:(q + 1) * 128],
                                        identity=ident[:D, :D])
                    nc.tensor.transpose(out=pt[:, q * 128 + D:(q + 1) * 128],
                                        in_=eT[:, q * 128:(q + 1) * 128],
                                        identity=ident[:D, :D])
                ot = p0.tile([128, CHUNK], F16, tag="ot")
                half = CHUNK // 2
                nc.vector.tensor_copy(out=ot[:, :half], in_=pt[:, :half])
                nc.scalar.copy(out=ot[:, half:], in_=pt[:, half:])
                nc.sync.dma_start(
                    out=tp_b[sl, :].rearrange("(q p) c -> p q c", p=128),
                    in_=ot[:].rearrange("p (q c) -> p q c", c=128),
                )

            # ---- AllGather the table, zero the sentinel pair --------------
            nc.gpsimd.collective_compute(
                "AllGather", mybir.AluOpType.bypass, replica_groups=GROUP,
                ins=[tp_b.ap().opt()], outs=[tp.ap().opt()])
            nc.sync.dma_start(out=tp[TH - 2:TH, :], in_=zrow[:])

            # ---- phase 1: emb pair-gathers, select, add -------------------
            col0 = 0
            for r, nb in enumerate(EMB_NB):
                for j0 in range(0, nb, gcols):
                    w = min(gcols, nb - j0)
                    cl, cr = col0 + j0, col0 + j0 + w
                    st = ps.tile([128, gcols * 4 * D], F16, tag="stag_e")
                    st3 = st[:, :w * 4 * D].rearrange(
                        "p (j c) -> p j c", c=4 * D)
                    nc.gpsimd.dma_gather(
                        out_ap=st3, in_ap=tp_pair,
                        idxs_ap=idx_e_sb[:, 8 * cl:8 * cr],
                        num_idxs=128 * w, num_idxs_reg=128 * w,
                        elem_size=4 * D, single_packet=False, queue_num=0)
                    sel = psel.tile([128, gcols * 2 * D], F16, tag="sel_e")
                    sv = sel[:, :w * 2 * D]
                    nc.scalar.copy(out=sv, in_=st3[:, :, 0:2 * D])
                    nc.vector.copy_predicated(
                        out=sv.rearrange("p (j c) -> p j c", c=2 * D),
                        mask=mask_e_sb[:, cl:cr].to_broadcast([128, w, 2 * D]),
                        data=st3[:, :, 2 * D:4 * D])
                    nc.vector.tensor_add(
                        out=acc_e[:, j0 * 128:(j0 + w) * 128],
                        in0=acc_e[:, j0 * 128:(j0 + w) * 128],
                        in1=sv)
                col0 += nb

            # ---- finals: v = num/den, l2norm, write out fp16 --------------
            acc3 = acc_e[:].rearrange("p (b c) -> p b c", c=128)
            num = acc3[:, :, 0:D]
            den = acc3[:, :, D:2 * D]
            nc.vector.tensor_scalar_max(den, den, 1e-30)
            nc.vector.reciprocal(den, den)
            v = pp.tile([128, EMB_NBLK * D], F32, tag="vfin")
            v3 = v[:].rearrange("p (b c) -> p b c", c=D)
            nc.vector.tensor_tensor(out=v3, in0=num, in1=den,
                                    op=mybir.AluOpType.mult)
            ssq = pp.tile([128, EMB_NBLK], F32, tag="ssq")
            for b in range(EMB_NBLK):
                sqs = p0.tile([128, D], F32, tag="sqscratch")
                nc.scalar.activation(
                    out=sqs[:], in_=v[:, b * D:(b + 1) * D],
                    func=mybir.ActivationFunctionType.Square,
                    accum_out=ssq[:, b:b + 1])
            nc.vector.tensor_scalar_max(ssq[:], ssq[:], 1e-24)
            nc.scalar.sqrt(out=ssq[:], in_=ssq[:])
            nc.vector.reciprocal(ssq[:], ssq[:])
            vo = pp.tile([128, EMB_NBLK * D], F16, tag="vfin16")
            for b in range(EMB_NBLK):
                nc.scalar.mul(out=vo[:, b * D:(b + 1) * D],
                              in_=v[:, b * D:(b + 1) * D],
                              mul=ssq[:, b:b + 1])
            oo = pp.tile([128, OFF_NBLK * D], F16, tag="off16")
            nc.vector.tensor_copy(out=oo[:], in_=acc_o[:])
            nc.sync.dma_start(out=emb_out[:], in_=vo[:])
            nc.sync.dma_start(out=off_out[:], in_=oo[:])

    nc.compile()
    return nc


# --------------------------------------------------------------------------
# top-level entry
# --------------------------------------------------------------------------

def _prepare(inputs, TH):
    sent_pair = (TH - 2) // 2
    h1 = np.asarray(inputs["head1"])
    t1 = np.asarray(inputs["tail1"])
    h2 = np.asarray(inputs["head2"])
    t2 = np.asarray(inputs["tail2"])

    m = h1 < NV
    emb_cores, EMB_NB, EMB_NBLK = _shard_and_rounds(
        h1[m], t1[m], NCORES, sent_pair)

    m1 = (h1 < NV) & (t1 >= NV)
    m2 = h2 < NV
    ho = np.concatenate([h1[m1], h2[m2]])
    to = np.concatenate([t1[m1], t2[m2]])
    off_cores, OFF_NB, OFF_NBLK = _shard_and_rounds(ho, to, NCORES, sent_pair)

    all_center = np.concatenate(
        [inputs["visit_center"], inputs["ccs_center"], inputs["icd_center"]], 0)
    all_offset = np.concatenate(
        [inputs["visit_offset"], inputs["ccs_offset"], inputs["icd_offset"]], 0)
    center_pad = np.zeros((TH, D), np.float16)
    center_pad[:len(all_center)] = all_center.astype(np.float16)
    offset_pad = np.zeros((TH, D), np.float16)
    offset_pad[:len(all_offset)] = all_offset.astype(np.float16)
    return dict(emb_cores=emb_cores, EMB_NB=EMB_NB, EMB_NBLK=EMB_NBLK,
                off_cores=off_cores, OFF_NB=OFF_NB, OFF_NBLK=OFF_NBLK,
                center16=center_pad, offset16=offset_pad)


def kernel(**inputs):
    TH = -(-NN // CHUNK) * CHUNK          # 57344
    SH = TH // NCORES
    prep = _prepare(inputs, TH)

    cfg = dict(TH=TH,
               EMB_NB=list(prep["EMB_NB"]), EMB_NBLK=prep["EMB_NBLK"],
               OFF_NB=list(prep["OFF_NB"]), OFF_NBLK=prep["OFF_NBLK"],
               gcols=12, stage_bufs=5)
    nc = _build_nc(cfg)

    common = dict(
        w1t=np.ascontiguousarray(np.asarray(inputs["att_w1"]).T),
        w2t=np.ascontiguousarray(np.asarray(inputs["att_w2"]).T),
        b1=np.asarray(inputs["att_b1"]).reshape(D, 1),
        b2=np.asarray(inputs["att_b2"]).reshape(D, 1),
    )
    in_maps = []
    for k in range(NCORES):
        m = dict(common)
        m["center_sh"] = prep["center16"][k * SH:(k + 1) * SH]
        m["offset_sh"] = prep["offset16"][k * SH:(k + 1) * SH]
        m["idx_e"] = prep["emb_cores"][k]["idx16"]
        m["idx_o"] = prep["off_cores"][k]["idx16"]
        m["mask_e"] = prep["emb_cores"][k]["mask"]
        m["mask_o"] = prep["off_cores"][k]["mask"]
        in_maps.append(m)

    res = run_bass_kernel_spmd(nc, in_maps, core_ids=list(range(NCORES)))
    _last_results["res"] = res
    _last_results["nc"] = nc
    _last_results["in_maps"] = in_maps

    emb = np.zeros((NV, D), np.float32)
    off = np.zeros((NV, D), np.float32)
    for k in range(NCORES):
        ce = prep["emb_cores"][k]
        co = prep["off_cores"][k]
        eo = res.results[k]["emb_out"].reshape(128, prep["EMB_NBLK"], D)
        oo = res.results[k]["off_out"].reshape(128, prep["OFF_NBLK"], D)
        eo = eo.transpose(1, 0, 2).reshape(-1, D).astype(np.float32)
        oo = oo.transpose(1, 0, 2).reshape(-1, D).astype(np.float32)
        emb[ce["nlo"] + ce["order"]] = eo[:ce["nhi"] - ce["nlo"]]
        off[co["nlo"] + co["order"]] = oo[:co["nhi"] - co["nlo"]]
    return emb, off


# revision 30
# speedup vs baseline: 7.6959x; 1.4423x over previous
"""Trainium2 Bass kernel for nn_BoxLM_1168231104949 (gnn_message_passing).

Contract: kernel(**inputs) takes the FULL unsharded inputs (as produced by
setup_inputs()) and returns the full output (visit_final_emb,
visit_final_offset), each [50000, 64] float32.

Math notes (validated against the reference in fp64/numpy):
  * lam == 1.0  =>  visit_final_emb == l2norm(center_net(all_center[tail1],
    head1, N_NODES)[:NV]); the graph-2 center_net contributes exactly 0.
  * logits are tiny (|l| < ~1) so the segment softmax is computed with a raw
    exp (no per-segment max subtraction): out = num/den with
    num = seg_sum(exp(l)*emb), den = seg_sum(exp(l)).
  * exp(l) depends only on the tail node, so it is precomputed per node into
    a table T[v] = [exp(l(v))*center(v) | exp(l(v))] (fp16, 128 ch) and the
    edge work reduces to row gathers + segment sums.
  * The five masked/clamped segment maxes for visit_final_offset collapse to
    one masked segment max over (graph1: tail>=NV) + (graph2: all) edges,
    clamped at 0 (the accumulator initialised to 0 provides the clamp, and
    relu commutes with max so raw offsets are gathered).

Distribution: edges are sorted by head on the host and sharded into 8
contiguous head ranges balanced by edge count - each core owns a disjoint
slice of output nodes.  Within a core, nodes are ordered by degree into
"slots"; round r gathers the r-th edge of every node with degree > r via one
bulk dma_gather (slot i -> partition i%128, block i//128 - exactly the
accumulator layout).  dma_gather indices are int16, so rows are fetched in
PAIRS (pair idx = tail//2 <= 28671) and the correct half is selected
on-chip with a host-provided parity mask.

Wall time through the axon tunnel is dominated by host<->device transfer
(~40 MB/s measured), so inputs are shipped minimally: the node tables are
SHARDED 1/8 per core in fp16 and reconstructed on-device with NeuronLink
AllGather (each core builds its slice of the exp table from its center
shard before the gather); dma_gather index buffers are shipped un-replicated
[16, 8*CT] and expanded to the required [128, 8*CT] layout on-device; the
outputs return as fp16.
"""

import numpy as np

import concourse.bacc as bacc
import concourse.bass as bass
import concourse.mybir as mybir
import concourse.tile as tile
from concourse.bass_utils import run_bass_kernel_spmd
from concourse.masks import make_identity

F32 = mybir.dt.float32
F16 = mybir.dt.float16
I16 = mybir.dt.int16
I8 = mybir.dt.int8

NV = 50000
NN = 57300
D = 64
NCORES = 8

CHUNK = 512        # table rows per phase-0 chunk
GCOLS = 25         # max 128-slot blocks per gather call

_last_results = {}
_KERNEL_OVERRIDES = {}     # experiment knobs (gcols/stage_bufs/nqueues/...)


# --------------------------------------------------------------------------
# host-side index preprocessing
# --------------------------------------------------------------------------

def _shard_and_rounds(heads, tails, ncores, sent_grp, pfac):
    """Sort edges by head, shard into contiguous node ranges balanced by edge
    count, order nodes by degree desc, emit per-round int16 group-index
    buffers (un-replicated dma_gather layout) + remainder masks.

    Rows are fetched in groups of pfac consecutive table rows per descriptor
    (idx = tail // pfac); mask holds tail % pfac for the on-chip select.

    Returns (cores, NB, NBLK).  cores[k]: nlo/nhi/order/idx16/mask.
    NB[r] = 128-slot blocks in round r (uniform across cores).
    """
    lg = pfac.bit_length() - 1
    deg = np.bincount(heads, minlength=NV)
    cum = np.cumsum(deg)
    total = int(cum[-1])
    bounds = [0]
    for k in range(1, ncores):
        bounds.append(int(np.searchsorted(cum, total * k / ncores)))
    bounds.append(NV)

    order_e = np.argsort(heads, kind="stable")
    t_s = tails[order_e]
    node_start = np.zeros(NV + 1, np.int64)
    node_start[1:] = cum

    cores = []
    for k in range(ncores):
        nlo, nhi = bounds[k], bounds[k + 1]
        ldeg = deg[nlo:nhi]
        order = np.argsort(-ldeg, kind="stable")
        cores.append(dict(nlo=nlo, nhi=nhi, order=order,
                          sorted_deg=ldeg[order]))
    R = max(int(c["sorted_deg"][0]) if len(c["sorted_deg"]) else 0
            for c in cores)
    NBLK = max(-(-(c["nhi"] - c["nlo"]) // 128) for c in cores)
    NB = []
    for r in range(R):
        cnt = max(int(np.searchsorted(-c["sorted_deg"], -r, side="left"))
                  for c in cores)
        NB.append(max(1, -(-cnt // 128)))
    CT = sum(NB)
    for c in cores:
        nlo = c["nlo"]
        # per-slot tail (sent = pfac*sent_grp for padding), slot-major/round
        pair = np.full((CT * 128,), sent_grp, np.int32)
        par = np.zeros((CT * 128,), np.int8)
        col0 = 0
        for r, nb in enumerate(NB):
            cnt_k = int(np.searchsorted(-c["sorted_deg"], -r, side="left"))
            s = np.arange(cnt_k)
            g = nlo + c["order"][s]
            tr = t_s[node_start[g] + r]
            pair[col0 * 128 + s] = tr >> lg
            par[col0 * 128 + s] = (tr & (pfac - 1)).astype(np.int8)
            col0 += nb
        # int16 dma_gather layout: per round section, slots wrapped into 16
        # partitions ([16, 8*nb], slot i at [i%16, i//16]); the x8 partition
        # replication the gather engine wants is done on-device.
        idx16 = np.empty((16, 8 * CT), np.int16)
        col0 = 0
        for r, nb in enumerate(NB):
            vals = pair[col0 * 128:(col0 + nb) * 128]
            sec = vals.reshape(8 * nb, 16).T.astype(np.int16)     # [16, 8nb]
            idx16[:, 8 * col0:8 * (col0 + nb)] = sec
            col0 += nb
        # remainder mask [128, CT]: slot j*128+p -> [p, col0+j], bit-packed
        # along cols: packed[:, j] bit lg*k+b = bit b of rem at col j*per+k
        mask = par.reshape(CT, 128).T.astype(np.uint8)            # [128, CT]
        per = 8 // lg
        CP = -(-CT // per)
        padm = np.zeros((128, CP * per), np.uint8)
        padm[:, :CT] = mask
        packed = np.zeros((128, CP), np.uint8)
        for k in range(per):
            packed |= padm[:, k::per] << (lg * k)
        c["idx16"] = idx16
        c["mask"] = packed.view(np.int8)
    return cores, NB, NBLK


# --------------------------------------------------------------------------
# device kernel builder
# --------------------------------------------------------------------------

def _build_nc(cfg):
    TH = cfg["TH"]
    SH = TH // NCORES
    EMB_NB, EMB_NBLK = cfg["EMB_NB"], cfg["EMB_NBLK"]
    OFF_NB, OFF_NBLK = cfg["OFF_NB"], cfg["OFF_NBLK"]
    CE = max(1, sum(EMB_NB))
    CO = max(1, sum(OFF_NB))
    NCH = SH // CHUNK
    gcols = cfg.get("gcols", GCOLS)
    stage_bufs = cfg.get("stage_bufs", 2)
    nq = cfg.get("nqueues", 2)
    single_packet = cfg.get("single_packet", False)
    pfac_e = cfg.get("pfac_e", 2)
    pfac_o = cfg.get("pfac_o", 2)
    lg_e = pfac_e.bit_length() - 1
    lg_o = pfac_o.bit_length() - 1
    GROUP = [list(range(NCORES))]

    nc = bacc.Bacc(None, target_bir_lowering=False, debug=False,
                   num_devices=NCORES, num_swdge_queues=nq)

    # packed masks: emb 1 bit/slot (8 cols/byte), off 2 bits/slot (4/byte)
    CEP = -(-CE // 8)
    COP = -(-CO // 4)

    center_sh = nc.dram_tensor("center_sh", [SH, D], F16, kind="ExternalInput")
    offset_sh = nc.dram_tensor("offset_sh", [SH, D], I8, kind="ExternalInput")
    w1t = nc.dram_tensor("w1t", [D, D], F32, kind="ExternalInput")
    w2t = nc.dram_tensor("w2t", [D, D], F32, kind="ExternalInput")
    b1 = nc.dram_tensor("b1", [D, 1], F32, kind="ExternalInput")
    b2 = nc.dram_tensor("b2", [D, 1], F32, kind="ExternalInput")
    idx_e = nc.dram_tensor("idx_e", [16, 8 * CE], I16, kind="ExternalInput")
    idx_o = nc.dram_tensor("idx_o", [16, 8 * CO], I16, kind="ExternalInput")
    mask_e = nc.dram_tensor("mask_e", [128, CEP], I8, kind="ExternalInput")
    mask_o = nc.dram_tensor("mask_o", [128, COP], I8, kind="ExternalInput")

    tp_b = nc.dram_tensor("tp_b", [SH, 2 * D], F16)     # local table shard
    tp = nc.dram_tensor("tp", [TH, 2 * D], F16)         # AllGather output
    off_b = nc.dram_tensor("off_b", [SH, D], I8)
    offf = nc.dram_tensor("offf", [TH, D], I8)

    emb_out = nc.dram_tensor("emb_out", [128, EMB_NBLK * D], I8,
                             kind="ExternalOutput")
    off_out = nc.dram_tensor("off_out", [128, OFF_NBLK * D], I8,
                             kind="ExternalOutput")

    tp_grp = tp[:].rearrange("(u f) c -> u (f c)", f=pfac_e)    # [TH/fe, fe*2D]
    off_grp = offf[:].rearrange("(u f) c -> u (f c)", f=pfac_o)  # [TH/fo, fo*D]

    with tile.TileContext(nc) as tc:
        with (
            tc.tile_pool(name="persist", bufs=1) as pp,
            tc.tile_pool(name="ph0", bufs=3) as p0,
            tc.tile_pool(name="ph0psum", bufs=2, space="PSUM") as pps,
            tc.tile_pool(name="stage", bufs=stage_bufs) as ps,
            tc.tile_pool(name="selp", bufs=2) as psel,
        ):
            # ---- offset table: bounce shard -> AllGather (early) -----------
            nc.sync.dma_start(out=off_b[:], in_=offset_sh[:])
            nc.gpsimd.collective_compute(
                "AllGather", mybir.AluOpType.bypass, replica_groups=GROUP,
                ins=[off_b.ap().opt()], outs=[offf.ap().opt()])

            # ---- constants -------------------------------------------------
            w1t_sb = pp.tile([D, D], F32, tag="w1t")
            w2t_sb = pp.tile([D, D], F32, tag="w2t")
            b1_sb = pp.tile([D, 1], F32, tag="b1")
            b2_sb = pp.tile([D, 1], F32, tag="b2")
            ident = pp.tile([128, 128], F32, tag="ident")
            zrow = pp.tile([pfac_e, 2 * D], F16, tag="zrow")
            nc.sync.dma_start(out=w1t_sb[:], in_=w1t[:])
            nc.sync.dma_start(out=w2t_sb[:], in_=w2t[:])
            nc.sync.dma_start(out=b1_sb[:], in_=b1[:])
            nc.sync.dma_start(out=b2_sb[:], in_=b2[:])
            make_identity(nc, ident[:])
            nc.vector.memset(zrow[:], 0.0)

            # ---- persistent phase-1 state ---------------------------------
            # idx buffers are shipped [16, 8*C] and replicated to the
            # [128, 8*C] layout dma_gather wants (8 copies along partitions).
            idx_e_sb = pp.tile([128, 8 * CE], I16, tag="idx_e")
            idx_o_sb = pp.tile([128, 8 * CO], I16, tag="idx_o")
            mask_e_sb = pp.tile([128, CEP], I8, tag="mask_e")
            mask_o_sb = pp.tile([128, COP], I8, tag="mask_o")
            acc_e = pp.tile([128, EMB_NBLK * 128], F32, tag="acc_e")
            acc_o = pp.tile([128, OFF_NBLK * D], I8, tag="acc_o")
            for r in range(8):
                nc.sync.dma_start(out=idx_e_sb[16 * r:16 * (r + 1), :],
                                  in_=idx_e[:])
                nc.sync.dma_start(out=idx_o_sb[16 * r:16 * (r + 1), :],
                                  in_=idx_o[:])
            nc.sync.dma_start(out=mask_e_sb[:], in_=mask_e[:])
            nc.sync.dma_start(out=mask_o_sb[:], in_=mask_o[:])
            nc.vector.memset(acc_e[:], 0.0)
            nc.vector.memset(acc_o[:], 0.0)

            # unpack bit-packed remainder masks into per-bit predicate planes
            # (device layout: plane b, slot col j*per+k <- packed[:, j] bit
            # lg*k+b; nonzero byte == predicate true)
            def bit_masks(packed_sb, CP, lg, tag):
                per = 8 // lg
                mb = []
                for b in range(lg):
                    t = pp.tile([128, CP * per], I8, tag=f"mb_{tag}{b}")
                    t3 = t[:].rearrange("p (j k) -> p j k", k=per)
                    for k in range(per):
                        nc.vector.tensor_scalar(
                            out=t3[:, :, k], in0=packed_sb[:],
                            scalar1=1 << (lg * k + b), scalar2=None,
                            op0=mybir.AluOpType.bitwise_and)
                    mb.append(t)
                return mb

            mb_e = bit_masks(mask_e_sb, CEP, lg_e, "e")
            mb_o = bit_masks(mask_o_sb, COP, lg_o, "o")

            # gather pfac rows per descriptor, select the true row with a
            # log2(pfac)-stage predicated cascade, fold into the accumulator
            no_gather = cfg.get("no_gather", False)
            no_select = cfg.get("no_select", False)

            def gather_path(NB_list, grp_ap, idx_sb, mb, base, lg, acc_fn,
                            tag, q0, nqs, dt):
                col0 = 0
                call = 0
                f = 1 << lg
                for r, nb in enumerate(NB_list):
                    for j0 in range(0, nb, gcols):
                        w = min(gcols, nb - j0)
                        cl, cr = col0 + j0, col0 + j0 + w
                        st = ps.tile([128, gcols * f * base], dt,
                                     tag=f"stag_{tag}")
                        cur = st[:, :w * f * base].rearrange(
                            "p (j c) -> p j c", c=f * base)
                        if not no_gather:
                            nc.gpsimd.dma_gather(
                                out_ap=cur, in_ap=grp_ap,
                                idxs_ap=idx_sb[:, 8 * cl:8 * cr],
                                num_idxs=128 * w, num_idxs_reg=128 * w,
                                elem_size=f * base,
                                single_packet=single_packet,
                                queue_num=q0 + (call % nqs))
                        elif call < stage_bufs:
                            nc.vector.memset(st[:], 0.0)
                        call += 1
                        if no_select:
                            continue
                        cur2 = None
                        for b in range(lg - 1, -1, -1):
                            half = (1 << b) * base
                            nt = psel.tile([128, gcols * half], dt,
                                           tag=f"sel_{tag}{b}")
                            cur2 = nt[:, :w * half]
                            nxt = cur2.rearrange("p (j c) -> p j c", c=half)
                            nc.scalar.copy(out=nxt, in_=cur[:, :, 0:half])
                            nc.vector.copy_predicated(
                                out=nxt,
                                mask=mb[b][:, cl:cr].to_broadcast(
                                    [128, w, half]),
                                data=cur[:, :, half:2 * half])
                            cur = nxt
                        acc_fn(cur2, j0, w)
                    col0 += nb

            # ---- offset path: gather fp16 offsets, select, max ------------
            # (emitted first: needs only the early AllGather, overlaps the
            # table build)
            def acc_off(sv, j0, w):
                nc.vector.tensor_tensor(
                    out=acc_o[:, j0 * D:(j0 + w) * D],
                    in0=acc_o[:, j0 * D:(j0 + w) * D],
                    in1=sv, op=mybir.AluOpType.max)

            gather_path(OFF_NB, off_grp, idx_o_sb, mb_o, D, lg_o, acc_off,
                        "o", nq // 2, nq - nq // 2, I8)

            # ---- phase 0: local shard of node table  tp[v] = [e*c | e] ----
            for ch in range(NCH):
                sl = slice(ch * CHUNK, (ch + 1) * CHUNK)
                ld16 = p0.tile([128, (CHUNK // 128) * D], F16, tag="ld16")
                nc.sync.dma_start(
                    out=ld16[:].rearrange("p (q d) -> p q d", d=D),
                    in_=center_sh[sl, :].rearrange("(q p) d -> p q d", p=128))
                ld32 = p0.tile([128, (CHUNK // 128) * D], F32, tag="ld32")
                nc.vector.tensor_copy(out=ld32[:], in_=ld16[:])
                ctp = pps.tile([D, CHUNK], F32, tag="ctp")
                for q in range(CHUNK // 128):
                    nc.tensor.transpose(out=ctp[:, q * 128:(q + 1) * 128],
                                        in_=ld32[:, q * D:(q + 1) * D],
                                        identity=ident[:])
                ct = p0.tile([D, CHUNK], F32, tag="ct")
                nc.scalar.copy(out=ct[:], in_=ctp[:])
                ph = pps.tile([D, CHUNK], F32, tag="ph")
                nc.tensor.matmul(out=ph[:], lhsT=w1t_sb[:], rhs=ct[:],
                                 start=True, stop=True)
                hT = p0.tile([D, CHUNK], F32, tag="hT")
                nc.scalar.activation(out=hT[:], in_=ph[:],
                                     func=mybir.ActivationFunctionType.Relu,
                                     bias=b1_sb[:])
                pl = pps.tile([D, CHUNK], F32, tag="pl")
                nc.tensor.matmul(out=pl[:], lhsT=w2t_sb[:], rhs=hT[:],
                                 start=True, stop=True)
                eT = p0.tile([D, CHUNK], F32, tag="eT")
                nc.scalar.activation(out=eT[:], in_=pl[:],
                                     func=mybir.ActivationFunctionType.Exp,
                                     bias=b2_sb[:])
                pT = p0.tile([D, CHUNK], F32, tag="pT")
                nc.vector.tensor_tensor(out=pT[:], in0=eT[:], in1=ct[:],
                                        op=mybir.AluOpType.mult)
                pt = pps.tile([128, CHUNK], F32, tag="pt")
                for q in range(CHUNK // 128):
                    nc.tensor.transpose(out=pt[:, q * 128:q * 128 + D],
                                        in_=pT[:, q * 128:(q + 1) * 128],
                                        identity=ident[:D, :D])
                    nc.tensor.transpose(out=pt[:, q * 128 + D:(q + 1) * 128],
                                        in_=eT[:, q * 128:(q + 1) * 128],
                                        identity=ident[:D, :D])
                ot = p0.tile([128, CHUNK], F16, tag="ot")
                half = CHUNK // 2
                nc.vector.tensor_copy(out=ot[:, :half], in_=pt[:, :half])
                nc.scalar.copy(out=ot[:, half:], in_=pt[:, half:])
                nc.sync.dma_start(
                    out=tp_b[sl, :].rearrange("(q p) c -> p q c", p=128),
                    in_=ot[:].rearrange("p (q c) -> p q c", c=128),
                )

            # ---- AllGather the table, zero the sentinel group -------------
            nc.gpsimd.collective_compute(
                "AllGather", mybir.AluOpType.bypass, replica_groups=GROUP,
                ins=[tp_b.ap().opt()], outs=[tp.ap().opt()])
            nc.sync.dma_start(out=tp[TH - pfac_e:TH, :], in_=zrow[:])

            # ---- phase 1: emb gathers, select, add ------------------------
            def acc_emb(sv, j0, w):
                nc.vector.tensor_add(
                    out=acc_e[:, j0 * 128:(j0 + w) * 128],
                    in0=acc_e[:, j0 * 128:(j0 + w) * 128],
                    in1=sv)

            gather_path(EMB_NB, tp_grp, idx_e_sb, mb_e, 2 * D, lg_e, acc_emb,
                        "e", 0, nq // 2, F16)

            # ---- finals: v = num/den, l2norm, write out fp16 --------------
            acc3 = acc_e[:].rearrange("p (b c) -> p b c", c=128)
            num = acc3[:, :, 0:D]
            den = acc3[:, :, D:2 * D]
            nc.vector.tensor_scalar_max(den, den, 1e-30)
            nc.vector.reciprocal(den, den)
            v = pp.tile([128, EMB_NBLK * D], F32, tag="vfin")
            v3 = v[:].rearrange("p (b c) -> p b c", c=D)
            nc.vector.tensor_tensor(out=v3, in0=num, in1=den,
                                    op=mybir.AluOpType.mult)
            ssq = pp.tile([128, EMB_NBLK], F32, tag="ssq")
            for b in range(EMB_NBLK):
                sqs = p0.tile([128, D], F32, tag="sqscratch")
                nc.scalar.activation(
                    out=sqs[:], in_=v[:, b * D:(b + 1) * D],
                    func=mybir.ActivationFunctionType.Square,
                    accum_out=ssq[:, b:b + 1])
            nc.vector.tensor_scalar_max(ssq[:], ssq[:], 1e-24)
            nc.scalar.sqrt(out=ssq[:], in_=ssq[:])
            nc.vector.reciprocal(ssq[:], ssq[:])
            # int8 output: fold the x127 quantisation scale into 1/norm
            # (|v/norm| <= 1 so the scaled values stay in [-127, 127])
            nc.vector.tensor_scalar_mul(ssq[:], ssq[:], 127.0)
            for b in range(EMB_NBLK):
                nc.scalar.mul(out=v[:, b * D:(b + 1) * D],
                              in_=v[:, b * D:(b + 1) * D],
                              mul=ssq[:, b:b + 1])
            nc.vector.tensor_scalar(
                out=v[:], in0=v[:], scalar1=127.0, scalar2=-127.0,
                op0=mybir.AluOpType.min, op1=mybir.AluOpType.max)
            vo = pp.tile([128, EMB_NBLK * D], I8, tag="vfin8")
            nc.vector.tensor_copy(out=vo[:], in_=v[:])
            nc.sync.dma_start(out=emb_out[:], in_=vo[:])
            nc.sync.dma_start(out=off_out[:], in_=acc_o[:])

    nc.compile()
    return nc


# --------------------------------------------------------------------------
# top-level entry
# --------------------------------------------------------------------------

def _prepare(inputs, TH, pfac_e=2, pfac_o=2):
    sent_e = (TH - pfac_e) // pfac_e
    sent_o = (TH - pfac_o) // pfac_o
    h1 = np.asarray(inputs["head1"])
    t1 = np.asarray(inputs["tail1"])
    h2 = np.asarray(inputs["head2"])
    t2 = np.asarray(inputs["tail2"])

    m = h1 < NV
    emb_cores, EMB_NB, EMB_NBLK = _shard_and_rounds(
        h1[m], t1[m], NCORES, sent_e, pfac_e)

    m1 = (h1 < NV) & (t1 >= NV)
    m2 = h2 < NV
    ho = np.concatenate([h1[m1], h2[m2]])
    to = np.concatenate([t1[m1], t2[m2]])
    off_cores, OFF_NB, OFF_NBLK = _shard_and_rounds(
        ho, to, NCORES, sent_o, pfac_o)

    all_center = np.concatenate(
        [inputs["visit_center"], inputs["ccs_center"], inputs["icd_center"]], 0)
    all_offset = np.concatenate(
        [inputs["visit_offset"], inputs["ccs_offset"], inputs["icd_offset"]],
        0).astype(np.float32)
    center_pad = np.zeros((TH, D), np.float16)
    center_pad[:len(all_center)] = all_center.astype(np.float16)
    # offsets feed a segment max (monotone), so int8 quantisation with a
    # global scale survives the max exactly; dequantised on the host
    M = max(float(np.abs(all_offset).max()), 1e-12)
    offset_pad = np.zeros((TH, D), np.int8)
    offset_pad[:len(all_offset)] = np.rint(
        all_offset * (127.0 / M)).astype(np.int8)
    return dict(emb_cores=emb_cores, EMB_NB=EMB_NB, EMB_NBLK=EMB_NBLK,
                off_cores=off_cores, OFF_NB=OFF_NB, OFF_NBLK=OFF_NBLK,
                center16=center_pad, offset8=offset_pad,
                off_scale=M / 127.0)


_nc_cache = {}


def kernel(**inputs):
    TH = -(-NN // CHUNK) * CHUNK          # 57344
    SH = TH // NCORES
    pfac_e = _KERNEL_OVERRIDES.get("pfac_e", 2)
    pfac_o = _KERNEL_OVERRIDES.get("pfac_o", 4)
    prep = _prepare(inputs, TH, pfac_e, pfac_o)

    cfg = dict(TH=TH,
               EMB_NB=list(prep["EMB_NB"]), EMB_NBLK=prep["EMB_NBLK"],
               OFF_NB=list(prep["OFF_NB"]), OFF_NBLK=prep["OFF_NBLK"],
               gcols=12, stage_bufs=5, pfac_e=pfac_e, pfac_o=pfac_o)
    cfg.update(_KERNEL_OVERRIDES)
    key = tuple(sorted((k, tuple(v) if isinstance(v, list) else v)
                       for k, v in cfg.items()))
    nc = _nc_cache.get(key)
    if nc is None:
        nc = _build_nc(cfg)
        _nc_cache[key] = nc

    common = dict(
        w1t=np.ascontiguousarray(np.asarray(inputs["att_w1"]).T),
        w2t=np.ascontiguousarray(np.asarray(inputs["att_w2"]).T),
        b1=np.asarray(inputs["att_b1"]).reshape(D, 1),
        b2=np.asarray(inputs["att_b2"]).reshape(D, 1),
    )
    in_maps = []
    for k in range(NCORES):
        m = dict(common)
        m["center_sh"] = prep["center16"][k * SH:(k + 1) * SH]
        m["offset_sh"] = prep["offset8"][k * SH:(k + 1) * SH]
        m["idx_e"] = prep["emb_cores"][k]["idx16"]
        m["idx_o"] = prep["off_cores"][k]["idx16"]
        m["mask_e"] = prep["emb_cores"][k]["mask"]
        m["mask_o"] = prep["off_cores"][k]["mask"]
        in_maps.append(m)

    res = run_bass_kernel_spmd(nc, in_maps, core_ids=list(range(NCORES)))
    _last_results["res"] = res
    _last_results["nc"] = nc
    _last_results["in_maps"] = in_maps

    emb = np.zeros((NV, D), np.float32)
    off = np.zeros((NV, D), np.float32)
    for k in range(NCORES):
        ce = prep["emb_cores"][k]
        co = prep["off_cores"][k]
        eo = res.results[k]["emb_out"].reshape(128, prep["EMB_NBLK"], D)
        oo = res.results[k]["off_out"].reshape(128, prep["OFF_NBLK"], D)
        eo = eo.transpose(1, 0, 2).reshape(-1, D).astype(np.float32) / 127.0
        oo = (oo.transpose(1, 0, 2).reshape(-1, D).astype(np.float32)
              * prep["off_scale"])
        emb[ce["nlo"] + ce["order"]] = eo[:ce["nhi"] - ce["nlo"]]
        off[co["nlo"] + co["order"]] = oo[:co["nhi"] - co["nlo"]]
    return emb, off


# revision 35
# speedup vs baseline: 8.0664x; 1.0481x over previous
"""Trainium2 Bass kernel for nn_BoxLM_1168231104949 (gnn_message_passing).

Contract: kernel(**inputs) takes the FULL unsharded inputs (as produced by
setup_inputs()) and returns the full output (visit_final_emb,
visit_final_offset), each [50000, 64] float32.

Math notes (validated against the reference in fp64/numpy):
  * lam == 1.0  =>  visit_final_emb == l2norm(center_net(all_center[tail1],
    head1, N_NODES)[:NV]); the graph-2 center_net contributes exactly 0.
  * logits are tiny (|l| < ~1) so the segment softmax is computed with a raw
    exp (no per-segment max subtraction): out = num/den with
    num = seg_sum(exp(l)*emb), den = seg_sum(exp(l)).
  * exp(l) depends only on the tail node, so it is precomputed per node into
    a table T[v] = [exp(l(v))*center(v) | exp(l(v))] (fp16, 128 ch) and the
    edge work reduces to row gathers + segment sums.
  * The five masked/clamped segment maxes for visit_final_offset collapse to
    one masked segment max over (graph1: tail>=NV) + (graph2: all) edges,
    clamped at 0 (the accumulator initialised to 0 provides the clamp, and
    relu commutes with max so raw offsets are gathered).

Distribution: edges are sorted by head on the host and sharded into 8
contiguous head ranges balanced by edge count - each core owns a disjoint
slice of output nodes.  Within a core, nodes are ordered by degree into
"slots"; round r gathers the r-th edge of every node with degree > r via one
bulk dma_gather (slot i -> partition i%128, block i//128 - exactly the
accumulator layout).  dma_gather indices are int16, so rows are fetched in
PAIRS (pair idx = tail//2 <= 28671) and the correct half is selected
on-chip with a host-provided parity mask.

Wall time through the axon tunnel is dominated by host<->device transfer
(~40 MB/s measured), so inputs are shipped minimally: the node tables are
SHARDED 1/8 per core in fp16 and reconstructed on-device with NeuronLink
AllGather (each core builds its slice of the exp table from its center
shard before the gather); dma_gather index buffers are shipped un-replicated
[16, 8*CT] and expanded to the required [128, 8*CT] layout on-device; the
outputs return as fp16.
"""

import numpy as np

import concourse.bacc as bacc
import concourse.bass as bass
import concourse.mybir as mybir
import concourse.tile as tile
from concourse.bass_utils import run_bass_kernel_spmd
from concourse.masks import make_identity

F32 = mybir.dt.float32
F16 = mybir.dt.float16
I16 = mybir.dt.int16
I8 = mybir.dt.int8

NV = 50000
NN = 57300
D = 64
NCORES = 8

CHUNK = 512        # table rows per phase-0 chunk
GCOLS = 25         # max 128-slot blocks per gather call

_last_results = {}
_KERNEL_OVERRIDES = {}     # experiment knobs (gcols/stage_bufs/nqueues/...)


# --------------------------------------------------------------------------
# host-side index preprocessing
# --------------------------------------------------------------------------

def _shard_and_rounds(heads, tails, ncores, sent_grp, pfac):
    """Sort edges by head, shard into contiguous node ranges balanced by edge
    count, order nodes by degree desc, emit per-round int16 group-index
    buffers (un-replicated dma_gather layout) + remainder masks.

    Rows are fetched in groups of pfac consecutive table rows per descriptor
    (idx = tail // pfac); mask holds tail % pfac for the on-chip select.

    Returns (cores, NB, NBLK).  cores[k]: nlo/nhi/order/idx16/mask.
    NB[r] = 128-slot blocks in round r (uniform across cores).
    """
    lg = pfac.bit_length() - 1
    deg = np.bincount(heads, minlength=NV)
    cum = np.cumsum(deg)
    total = int(cum[-1])
    bounds = [0]
    for k in range(1, ncores):
        bounds.append(int(np.searchsorted(cum, total * k / ncores)))
    bounds.append(NV)

    order_e = np.argsort(heads, kind="stable")
    t_s = tails[order_e]
    node_start = np.zeros(NV + 1, np.int64)
    node_start[1:] = cum

    cores = []
    for k in range(ncores):
        nlo, nhi = bounds[k], bounds[k + 1]
        ldeg = deg[nlo:nhi]
        order = np.argsort(-ldeg, kind="stable")
        cores.append(dict(nlo=nlo, nhi=nhi, order=order,
                          sorted_deg=ldeg[order]))
    R = max(int(c["sorted_deg"][0]) if len(c["sorted_deg"]) else 0
            for c in cores)
    NBLK = max(-(-(c["nhi"] - c["nlo"]) // 128) for c in cores)
    NB = []
    for r in range(R):
        cnt = max(int(np.searchsorted(-c["sorted_deg"], -r, side="left"))
                  for c in cores)
        NB.append(max(1, -(-cnt // 128)))
    CT = sum(NB)
    for c in cores:
        nlo = c["nlo"]
        # per-slot tail (sent = pfac*sent_grp for padding), slot-major/round
        pair = np.full((CT * 128,), sent_grp, np.int32)
        par = np.zeros((CT * 128,), np.int8)
        col0 = 0
        for r, nb in enumerate(NB):
            cnt_k = int(np.searchsorted(-c["sorted_deg"], -r, side="left"))
            s = np.arange(cnt_k)
            g = nlo + c["order"][s]
            tr = t_s[node_start[g] + r]
            pair[col0 * 128 + s] = tr >> lg
            par[col0 * 128 + s] = (tr & (pfac - 1)).astype(np.int8)
            col0 += nb
        # int16 dma_gather layout: per round section, slots wrapped into 16
        # partitions ([16, 8*nb], slot i at [i%16, i//16]); the x8 partition
        # replication the gather engine wants is done on-device.
        idx16 = np.empty((16, 8 * CT), np.int16)
        col0 = 0
        for r, nb in enumerate(NB):
            vals = pair[col0 * 128:(col0 + nb) * 128]
            sec = vals.reshape(8 * nb, 16).T.astype(np.int16)     # [16, 8nb]
            idx16[:, 8 * col0:8 * (col0 + nb)] = sec
            col0 += nb
        # remainder mask [128, CT]: slot j*128+p -> [p, col0+j], bit-packed
        # along cols: packed[:, j] bit lg*k+b = bit b of rem at col j*per+k
        mask = par.reshape(CT, 128).T.astype(np.uint8)            # [128, CT]
        per = 8 // lg
        CP = -(-CT // per)
        padm = np.zeros((128, CP * per), np.uint8)
        padm[:, :CT] = mask
        packed = np.zeros((128, CP), np.uint8)
        for k in range(per):
            packed |= padm[:, k::per] << (lg * k)
        c["idx16"] = idx16
        c["mask"] = packed.view(np.int8)
    return cores, NB, NBLK


# --------------------------------------------------------------------------
# device kernel builder
# --------------------------------------------------------------------------

def _build_nc(cfg):
    TH = cfg["TH"]
    SH = TH // NCORES
    EMB_NB, EMB_NBLK = cfg["EMB_NB"], cfg["EMB_NBLK"]
    OFF_NB, OFF_NBLK = cfg["OFF_NB"], cfg["OFF_NBLK"]
    CE = max(1, sum(EMB_NB))
    CO = max(1, sum(OFF_NB))
    NCH = SH // CHUNK
    gcols = cfg.get("gcols", GCOLS)
    stage_bufs = cfg.get("stage_bufs", 2)
    nq = cfg.get("nqueues", 2)
    single_packet = cfg.get("single_packet", False)
    pfac_e = cfg.get("pfac_e", 2)
    pfac_o = cfg.get("pfac_o", 2)
    lg_e = pfac_e.bit_length() - 1
    lg_o = pfac_o.bit_length() - 1
    GROUP = [list(range(NCORES))]

    nc = bacc.Bacc(None, target_bir_lowering=False, debug=False,
                   num_devices=NCORES, num_swdge_queues=nq)

    # packed masks: emb 1 bit/slot (8 cols/byte), off 2 bits/slot (4/byte)
    CEP = -(-CE // 8)
    COP = -(-CO // 4)

    center_sh = nc.dram_tensor("center_sh", [SH, D], I8, kind="ExternalInput")
    cscale_sh = nc.dram_tensor("cscale_sh", [SH // 128, 128], F32,
                               kind="ExternalInput")
    offset_sh = nc.dram_tensor("offset_sh", [SH, D], I8, kind="ExternalInput")
    w1t = nc.dram_tensor("w1t", [D, D], F32, kind="ExternalInput")
    w2t = nc.dram_tensor("w2t", [D, D], F32, kind="ExternalInput")
    b1 = nc.dram_tensor("b1", [D, 1], F32, kind="ExternalInput")
    b2 = nc.dram_tensor("b2", [D, 1], F32, kind="ExternalInput")
    idx_e = nc.dram_tensor("idx_e", [16, 8 * CE], I16, kind="ExternalInput")
    idx_o = nc.dram_tensor("idx_o", [16, 8 * CO], I16, kind="ExternalInput")
    mask_e = nc.dram_tensor("mask_e", [128, CEP], I8, kind="ExternalInput")
    mask_o = nc.dram_tensor("mask_o", [128, COP], I8, kind="ExternalInput")

    tp_b = nc.dram_tensor("tp_b", [SH, 2 * D], F16)     # local table shard
    tp = nc.dram_tensor("tp", [TH, 2 * D], F16)         # AllGather output
    off_b = nc.dram_tensor("off_b", [SH, D], I8)
    offf = nc.dram_tensor("offf", [TH, D], I8)

    emb_out = nc.dram_tensor("emb_out", [128, EMB_NBLK * D], I8,
                             kind="ExternalOutput")
    off_out = nc.dram_tensor("off_out", [128, OFF_NBLK * D], I8,
                             kind="ExternalOutput")

    tp_grp = tp[:].rearrange("(u f) c -> u (f c)", f=pfac_e)    # [TH/fe, fe*2D]
    off_grp = offf[:].rearrange("(u f) c -> u (f c)", f=pfac_o)  # [TH/fo, fo*D]

    with tile.TileContext(nc) as tc:
        with (
            tc.tile_pool(name="persist", bufs=1) as pp,
            tc.tile_pool(name="ph0", bufs=3) as p0,
            tc.tile_pool(name="ph0psum", bufs=2, space="PSUM") as pps,
            tc.tile_pool(name="stage", bufs=stage_bufs) as ps,
            tc.tile_pool(name="selp", bufs=2) as psel,
        ):
            # ---- offset table: bounce shard -> AllGather (early) -----------
            nc.sync.dma_start(out=off_b[:], in_=offset_sh[:])
            nc.gpsimd.collective_compute(
                "AllGather", mybir.AluOpType.bypass, replica_groups=GROUP,
                ins=[off_b.ap().opt()], outs=[offf.ap().opt()])

            # ---- constants -------------------------------------------------
            w1t_sb = pp.tile([D, D], F32, tag="w1t")
            w2t_sb = pp.tile([D, D], F32, tag="w2t")
            b1_sb = pp.tile([D, 1], F32, tag="b1")
            b2_sb = pp.tile([D, 1], F32, tag="b2")
            ident = pp.tile([128, 128], F32, tag="ident")
            zrow = pp.tile([pfac_e, 2 * D], F16, tag="zrow")
            nc.sync.dma_start(out=w1t_sb[:], in_=w1t[:])
            nc.sync.dma_start(out=w2t_sb[:], in_=w2t[:])
            nc.sync.dma_start(out=b1_sb[:], in_=b1[:])
            nc.sync.dma_start(out=b2_sb[:], in_=b2[:])
            make_identity(nc, ident[:])
            nc.vector.memset(zrow[:], 0.0)

            # ---- persistent phase-1 state ---------------------------------
            # idx buffers are shipped [16, 8*C] and replicated to the
            # [128, 8*C] layout dma_gather wants (8 copies along partitions).
            idx_e_sb = pp.tile([128, 8 * CE], I16, tag="idx_e")
            idx_o_sb = pp.tile([128, 8 * CO], I16, tag="idx_o")
            mask_e_sb = pp.tile([128, CEP], I8, tag="mask_e")
            mask_o_sb = pp.tile([128, COP], I8, tag="mask_o")
            acc_e = pp.tile([128, EMB_NBLK * 128], F32, tag="acc_e")
            acc_o = pp.tile([128, OFF_NBLK * D], I8, tag="acc_o")
            for r in range(8):
                nc.sync.dma_start(out=idx_e_sb[16 * r:16 * (r + 1), :],
                                  in_=idx_e[:])
                nc.sync.dma_start(out=idx_o_sb[16 * r:16 * (r + 1), :],
                                  in_=idx_o[:])
            nc.sync.dma_start(out=mask_e_sb[:], in_=mask_e[:])
            nc.sync.dma_start(out=mask_o_sb[:], in_=mask_o[:])
            nc.vector.memset(acc_e[:], 0.0)
            nc.vector.memset(acc_o[:], 0.0)

            # unpack bit-packed remainder masks into per-bit predicate planes
            # (device layout: plane b, slot col j*per+k <- packed[:, j] bit
            # lg*k+b; nonzero byte == predicate true)
            def bit_masks(packed_sb, CP, lg, tag):
                per = 8 // lg
                mb = []
                for b in range(lg):
                    t = pp.tile([128, CP * per], I8, tag=f"mb_{tag}{b}")
                    t3 = t[:].rearrange("p (j k) -> p j k", k=per)
                    for k in range(per):
                        nc.vector.tensor_scalar(
                            out=t3[:, :, k], in0=packed_sb[:],
                            scalar1=1 << (lg * k + b), scalar2=None,
                            op0=mybir.AluOpType.bitwise_and)
                    mb.append(t)
                return mb

            mb_e = bit_masks(mask_e_sb, CEP, lg_e, "e")
            mb_o = bit_masks(mask_o_sb, COP, lg_o, "o")

            # gather pfac rows per descriptor, select the true row with a
            # log2(pfac)-stage predicated cascade, fold into the accumulator
            no_gather = cfg.get("no_gather", False)
            no_select = cfg.get("no_select", False)

            def gather_path(NB_list, grp_ap, idx_sb, mb, base, lg, acc_fn,
                            tag, q0, nqs, dt):
                col0 = 0
                call = 0
                f = 1 << lg
                for r, nb in enumerate(NB_list):
                    for j0 in range(0, nb, gcols):
                        w = min(gcols, nb - j0)
                        cl, cr = col0 + j0, col0 + j0 + w
                        st = ps.tile([128, gcols * f * base], dt,
                                     tag=f"stag_{tag}")
                        cur = st[:, :w * f * base].rearrange(
                            "p (j c) -> p j c", c=f * base)
                        if not no_gather:
                            nc.gpsimd.dma_gather(
                                out_ap=cur, in_ap=grp_ap,
                                idxs_ap=idx_sb[:, 8 * cl:8 * cr],
                                num_idxs=128 * w, num_idxs_reg=128 * w,
                                elem_size=f * base,
                                single_packet=single_packet,
                                queue_num=q0 + (call % nqs))
                        elif call < stage_bufs:
                            nc.vector.memset(st[:], 0.0)
                        call += 1
                        if no_select:
                            continue
                        cur2 = None
                        for b in range(lg - 1, -1, -1):
                            half = (1 << b) * base
                            nt = psel.tile([128, gcols * half], dt,
                                           tag=f"sel_{tag}{b}")
                            cur2 = nt[:, :w * half]
                            nxt = cur2.rearrange("p (j c) -> p j c", c=half)
                            nc.scalar.copy(out=nxt, in_=cur[:, :, 0:half])
                            nc.vector.copy_predicated(
                                out=nxt,
                                mask=mb[b][:, cl:cr].to_broadcast(
                                    [128, w, half]),
                                data=cur[:, :, half:2 * half])
                            cur = nxt
                        acc_fn(cur2, j0, w)
                    col0 += nb

            # ---- offset path: gather fp16 offsets, select, max ------------
            # (emitted first: needs only the early AllGather, overlaps the
            # table build)
            def acc_off(sv, j0, w):
                nc.vector.tensor_tensor(
                    out=acc_o[:, j0 * D:(j0 + w) * D],
                    in0=acc_o[:, j0 * D:(j0 + w) * D],
                    in1=sv, op=mybir.AluOpType.max)

            gather_path(OFF_NB, off_grp, idx_o_sb, mb_o, D, lg_o, acc_off,
                        "o", nq // 2, nq - nq // 2, I8)

            # ---- phase 0: local shard of node table  tp[v] = [e*c | e] ----
            # center shard arrives int8 with per-row scales; dequantise on
            # load (row ch*512 + q*128 + p -> ld[p, q*D:(q+1)*D])
            csc = pp.tile([128, SH // 128], F32, tag="csc")
            nc.sync.dma_start(out=csc[:],
                              in_=cscale_sh[:].rearrange("g p -> p g"))
            for ch in range(NCH):
                sl = slice(ch * CHUNK, (ch + 1) * CHUNK)
                ld8 = p0.tile([128, (CHUNK // 128) * D], I8, tag="ld8")
                nc.sync.dma_start(
                    out=ld8[:].rearrange("p (q d) -> p q d", d=D),
                    in_=center_sh[sl, :].rearrange("(q p) d -> p q d", p=128))
                ld32 = p0.tile([128, (CHUNK // 128) * D], F32, tag="ld32")
                nc.vector.tensor_copy(out=ld32[:], in_=ld8[:])
                for q in range(CHUNK // 128):
                    nc.scalar.mul(out=ld32[:, q * D:(q + 1) * D],
                                  in_=ld32[:, q * D:(q + 1) * D],
                                  mul=csc[:, ch * 4 + q:ch * 4 + q + 1])
                ctp = pps.tile([D, CHUNK], F32, tag="ctp")
                for q in range(CHUNK // 128):
                    nc.tensor.transpose(out=ctp[:, q * 128:(q + 1) * 128],
                                        in_=ld32[:, q * D:(q + 1) * D],
                                        identity=ident[:])
                ct = p0.tile([D, CHUNK], F32, tag="ct")
                nc.scalar.copy(out=ct[:], in_=ctp[:])
                ph = pps.tile([D, CHUNK], F32, tag="ph")
                nc.tensor.matmul(out=ph[:], lhsT=w1t_sb[:], rhs=ct[:],
                                 start=True, stop=True)
                hT = p0.tile([D, CHUNK], F32, tag="hT")
                nc.scalar.activation(out=hT[:], in_=ph[:],
                                     func=mybir.ActivationFunctionType.Relu,
                                     bias=b1_sb[:])
                pl = pps.tile([D, CHUNK], F32, tag="pl")
                nc.tensor.matmul(out=pl[:], lhsT=w2t_sb[:], rhs=hT[:],
                                 start=True, stop=True)
                eT = p0.tile([D, CHUNK], F32, tag="eT")
                nc.scalar.activation(out=eT[:], in_=pl[:],
                                     func=mybir.ActivationFunctionType.Exp,
                                     bias=b2_sb[:])
                pT = p0.tile([D, CHUNK], F32, tag="pT")
                nc.vector.tensor_tensor(out=pT[:], in0=eT[:], in1=ct[:],
                                        op=mybir.AluOpType.mult)
                pt = pps.tile([128, CHUNK], F32, tag="pt")
                for q in range(CHUNK // 128):
                    nc.tensor.transpose(out=pt[:, q * 128:q * 128 + D],
                                        in_=pT[:, q * 128:(q + 1) * 128],
                                        identity=ident[:D, :D])
                    nc.tensor.transpose(out=pt[:, q * 128 + D:(q + 1) * 128],
                                        in_=eT[:, q * 128:(q + 1) * 128],
                                        identity=ident[:D, :D])
                ot = p0.tile([128, CHUNK], F16, tag="ot")
                half = CHUNK // 2
                nc.vector.tensor_copy(out=ot[:, :half], in_=pt[:, :half])
                nc.scalar.copy(out=ot[:, half:], in_=pt[:, half:])
                nc.sync.dma_start(
                    out=tp_b[sl, :].rearrange("(q p) c -> p q c", p=128),
                    in_=ot[:].rearrange("p (q c) -> p q c", c=128),
                )

            # ---- AllGather the table, zero the sentinel group -------------
            nc.gpsimd.collective_compute(
                "AllGather", mybir.AluOpType.bypass, replica_groups=GROUP,
                ins=[tp_b.ap().opt()], outs=[tp.ap().opt()])
            nc.sync.dma_start(out=tp[TH - pfac_e:TH, :], in_=zrow[:])

            # ---- phase 1: emb gathers, select, add ------------------------
            def acc_emb(sv, j0, w):
                nc.vector.tensor_add(
                    out=acc_e[:, j0 * 128:(j0 + w) * 128],
                    in0=acc_e[:, j0 * 128:(j0 + w) * 128],
                    in1=sv)

            gather_path(EMB_NB, tp_grp, idx_e_sb, mb_e, 2 * D, lg_e, acc_emb,
                        "e", 0, nq // 2, F16)

            # ---- finals: v = num/den, l2norm, write out fp16 --------------
            acc3 = acc_e[:].rearrange("p (b c) -> p b c", c=128)
            num = acc3[:, :, 0:D]
            den = acc3[:, :, D:2 * D]
            nc.vector.tensor_scalar_max(den, den, 1e-30)
            nc.vector.reciprocal(den, den)
            v = pp.tile([128, EMB_NBLK * D], F32, tag="vfin")
            v3 = v[:].rearrange("p (b c) -> p b c", c=D)
            nc.vector.tensor_tensor(out=v3, in0=num, in1=den,
                                    op=mybir.AluOpType.mult)
            ssq = pp.tile([128, EMB_NBLK], F32, tag="ssq")
            for b in range(EMB_NBLK):
                sqs = p0.tile([128, D], F32, tag="sqscratch")
                nc.scalar.activation(
                    out=sqs[:], in_=v[:, b * D:(b + 1) * D],
                    func=mybir.ActivationFunctionType.Square,
                    accum_out=ssq[:, b:b + 1])
            nc.vector.tensor_scalar_max(ssq[:], ssq[:], 1e-24)
            nc.scalar.sqrt(out=ssq[:], in_=ssq[:])
            nc.vector.reciprocal(ssq[:], ssq[:])
            # int8 output: fold the x127 quantisation scale into 1/norm
            # (|v/norm| <= 1 so the scaled values stay in [-127, 127])
            nc.vector.tensor_scalar_mul(ssq[:], ssq[:], 127.0)
            for b in range(EMB_NBLK):
                nc.scalar.mul(out=v[:, b * D:(b + 1) * D],
                              in_=v[:, b * D:(b + 1) * D],
                              mul=ssq[:, b:b + 1])
            nc.vector.tensor_scalar(
                out=v[:], in0=v[:], scalar1=127.0, scalar2=-127.0,
                op0=mybir.AluOpType.min, op1=mybir.AluOpType.max)
            vo = pp.tile([128, EMB_NBLK * D], I8, tag="vfin8")
            nc.vector.tensor_copy(out=vo[:], in_=v[:])
            nc.sync.dma_start(out=emb_out[:], in_=vo[:])
            nc.sync.dma_start(out=off_out[:], in_=acc_o[:])

    nc.compile()
    return nc


# --------------------------------------------------------------------------
# top-level entry
# --------------------------------------------------------------------------

def _prepare(inputs, TH, pfac_e=2, pfac_o=2):
    sent_e = (TH - pfac_e) // pfac_e
    sent_o = (TH - pfac_o) // pfac_o
    h1 = np.asarray(inputs["head1"])
    t1 = np.asarray(inputs["tail1"])
    h2 = np.asarray(inputs["head2"])
    t2 = np.asarray(inputs["tail2"])

    m = h1 < NV
    emb_cores, EMB_NB, EMB_NBLK = _shard_and_rounds(
        h1[m], t1[m], NCORES, sent_e, pfac_e)

    m1 = (h1 < NV) & (t1 >= NV)
    m2 = h2 < NV
    ho = np.concatenate([h1[m1], h2[m2]])
    to = np.concatenate([t1[m1], t2[m2]])
    off_cores, OFF_NB, OFF_NBLK = _shard_and_rounds(
        ho, to, NCORES, sent_o, pfac_o)

    all_center = np.concatenate(
        [inputs["visit_center"], inputs["ccs_center"], inputs["icd_center"]],
        0).astype(np.float32)
    all_offset = np.concatenate(
        [inputs["visit_offset"], inputs["ccs_offset"], inputs["icd_offset"]],
        0).astype(np.float32)
    # center rows int8-quantised with per-row scale, dequantised on device
    cfull = np.zeros((TH, D), np.float32)
    cfull[:len(all_center)] = all_center
    crow = np.abs(cfull).max(axis=1)
    crow = np.maximum(crow, 1e-12)
    center_pad = np.rint(cfull * (127.0 / crow[:, None])).astype(np.int8)
    cscale = (crow / 127.0).astype(np.float32)
    # offsets feed a segment max (monotone), so int8 quantisation with a
    # global scale survives the max exactly; dequantised on the host
    M = max(float(np.abs(all_offset).max()), 1e-12)
    offset_pad = np.zeros((TH, D), np.int8)
    offset_pad[:len(all_offset)] = np.rint(
        all_offset * (127.0 / M)).astype(np.int8)
    return dict(emb_cores=emb_cores, EMB_NB=EMB_NB, EMB_NBLK=EMB_NBLK,
                off_cores=off_cores, OFF_NB=OFF_NB, OFF_NBLK=OFF_NBLK,
                center8=center_pad, cscale=cscale, offset8=offset_pad,
                off_scale=M / 127.0)


_nc_cache = {}


def kernel(**inputs):
    TH = -(-NN // CHUNK) * CHUNK          # 57344
    SH = TH // NCORES
    pfac_e = _KERNEL_OVERRIDES.get("pfac_e", 2)
    pfac_o = _KERNEL_OVERRIDES.get("pfac_o", 4)
    prep = _prepare(inputs, TH, pfac_e, pfac_o)

    cfg = dict(TH=TH,
               EMB_NB=list(prep["EMB_NB"]), EMB_NBLK=prep["EMB_NBLK"],
               OFF_NB=list(prep["OFF_NB"]), OFF_NBLK=prep["OFF_NBLK"],
               gcols=12, stage_bufs=5, pfac_e=pfac_e, pfac_o=pfac_o)
    cfg.update(_KERNEL_OVERRIDES)
    key = tuple(sorted((k, tuple(v) if isinstance(v, list) else v)
                       for k, v in cfg.items()))
    nc = _nc_cache.get(key)
    if nc is None:
        nc = _build_nc(cfg)
        _nc_cache[key] = nc

    common = dict(
        w1t=np.ascontiguousarray(np.asarray(inputs["att_w1"]).T),
        w2t=np.ascontiguousarray(np.asarray(inputs["att_w2"]).T),
        b1=np.asarray(inputs["att_b1"]).reshape(D, 1),
        b2=np.asarray(inputs["att_b2"]).reshape(D, 1),
    )
    in_maps = []
    for k in range(NCORES):
        m = dict(common)
        m["center_sh"] = prep["center8"][k * SH:(k + 1) * SH]
        m["cscale_sh"] = prep["cscale"][k * SH:(k + 1) * SH].reshape(
            SH // 128, 128)
        m["offset_sh"] = prep["offset8"][k * SH:(k + 1) * SH]
        m["idx_e"] = prep["emb_cores"][k]["idx16"]
        m["idx_o"] = prep["off_cores"][k]["idx16"]
        m["mask_e"] = prep["emb_cores"][k]["mask"]
        m["mask_o"] = prep["off_cores"][k]["mask"]
        in_maps.append(m)

    res = run_bass_kernel_spmd(nc, in_maps, core_ids=list(range(NCORES)))
    _last_results["res"] = res
    _last_results["nc"] = nc
    _last_results["in_maps"] = in_maps

    emb = np.zeros((NV, D), np.float32)
    off = np.zeros((NV, D), np.float32)
    for k in range(NCORES):
        ce = prep["emb_cores"][k]
        co = prep["off_cores"][k]
        eo = res.results[k]["emb_out"].reshape(128, prep["EMB_NBLK"], D)
        oo = res.results[k]["off_out"].reshape(128, prep["OFF_NBLK"], D)
        eo = eo.transpose(1, 0, 2).reshape(-1, D).astype(np.float32) / 127.0
        oo = (oo.transpose(1, 0, 2).reshape(-1, D).astype(np.float32)
              * prep["off_scale"])
        emb[ce["nlo"] + ce["order"]] = eo[:ce["nhi"] - ce["nlo"]]
        off[co["nlo"] + co["order"]] = oo[:co["nhi"] - co["nlo"]]
    return emb, off


# revision 43
# speedup vs baseline: 8.4853x; 1.0519x over previous
"""Trainium2 Bass kernel for nn_BoxLM_1168231104949 (gnn_message_passing).

Contract: kernel(**inputs) takes the FULL unsharded inputs (as produced by
setup_inputs()) and returns the full output (visit_final_emb,
visit_final_offset), each [50000, 64] float32.

Math notes (validated against the reference in fp64/numpy):
  * lam == 1.0  =>  visit_final_emb == l2norm(center_net(all_center[tail1],
    head1, N_NODES)[:NV]); the graph-2 center_net contributes exactly 0.
  * logits are tiny (|l| < ~1) so the segment softmax is computed with a raw
    exp (no per-segment max subtraction): out = num/den with
    num = seg_sum(exp(l)*emb), den = seg_sum(exp(l)).
  * exp(l) depends only on the tail node, so it is precomputed per node into
    a table T[v] = [exp(l(v))*center(v) | exp(l(v))] (fp16, 128 ch) and the
    edge work reduces to row gathers + segment sums.
  * The five masked/clamped segment maxes for visit_final_offset collapse to
    one masked segment max over (graph1: tail>=NV) + (graph2: all) edges,
    clamped at 0 (the accumulator initialised to 0 provides the clamp, and
    relu commutes with max so raw offsets are gathered).

Distribution: edges are sorted by head on the host and sharded into 8
contiguous head ranges balanced by edge count - each core owns a disjoint
slice of output nodes.  Within a core, nodes are ordered by degree into
"slots"; round r gathers the r-th edge of every node with degree > r via one
bulk dma_gather (slot i -> partition i%128, block i//128 - exactly the
accumulator layout).  dma_gather indices are int16, so rows are fetched in
PAIRS (pair idx = tail//2 <= 28671) and the correct half is selected
on-chip with a host-provided parity mask.

Wall time through the axon tunnel is dominated by host<->device transfer
(~40 MB/s measured), so inputs are shipped minimally: the node tables are
SHARDED 1/8 per core in fp16 and reconstructed on-device with NeuronLink
AllGather (each core builds its slice of the exp table from its center
shard before the gather); dma_gather index buffers are shipped un-replicated
[16, 8*CT] and expanded to the required [128, 8*CT] layout on-device; the
outputs return as fp16.
"""

import numpy as np

import concourse.bacc as bacc
import concourse.bass as bass
import concourse.mybir as mybir
import concourse.tile as tile
from concourse.bass_utils import run_bass_kernel_spmd
from concourse.masks import make_identity

F32 = mybir.dt.float32
F16 = mybir.dt.float16
I16 = mybir.dt.int16
I8 = mybir.dt.int8

NV = 50000
NN = 57300
D = 64
NCORES = 8

CHUNK = 512        # table rows per phase-0 chunk
GCOLS = 25         # max 128-slot blocks per gather call

_last_results = {}
_KERNEL_OVERRIDES = {}     # experiment knobs (gcols/stage_bufs/nqueues/...)


# --------------------------------------------------------------------------
# host-side index preprocessing
# --------------------------------------------------------------------------

def _shard_and_rounds(heads, tails, ncores, sent_grp, pfac):
    """Sort edges by head, shard into contiguous node ranges balanced by edge
    count, order nodes by degree desc, emit per-round int16 group-index
    buffers (un-replicated dma_gather layout) + remainder masks.

    Rows are fetched in groups of pfac consecutive table rows per descriptor
    (idx = tail // pfac); mask holds tail % pfac for the on-chip select.

    Returns (cores, NB, NBLK).  cores[k]: nlo/nhi/order/idx16/mask.
    NB[r] = 128-slot blocks in round r (uniform across cores).
    """
    lg = pfac.bit_length() - 1
    deg = np.bincount(heads, minlength=NV)
    cum = np.cumsum(deg)
    total = int(cum[-1])
    bounds = [0]
    for k in range(1, ncores):
        bounds.append(int(np.searchsorted(cum, total * k / ncores)))
    bounds.append(NV)

    order_e = np.argsort(heads, kind="stable")
    t_s = tails[order_e]
    node_start = np.zeros(NV + 1, np.int64)
    node_start[1:] = cum

    cores = []
    for k in range(ncores):
        nlo, nhi = bounds[k], bounds[k + 1]
        ldeg = deg[nlo:nhi]
        order = np.argsort(-ldeg, kind="stable")
        cores.append(dict(nlo=nlo, nhi=nhi, order=order,
                          sorted_deg=ldeg[order]))
    R = max(int(c["sorted_deg"][0]) if len(c["sorted_deg"]) else 0
            for c in cores)
    NBLK = max(-(-(c["nhi"] - c["nlo"]) // 128) for c in cores)
    NB = []
    for r in range(R):
        cnt = max(int(np.searchsorted(-c["sorted_deg"], -r, side="left"))
                  for c in cores)
        NB.append(max(1, -(-cnt // 128)))
    CT = sum(NB)
    for c in cores:
        nlo = c["nlo"]
        # per-slot tail (sent = pfac*sent_grp for padding), slot-major/round
        pair = np.full((CT * 128,), sent_grp, np.int32)
        par = np.zeros((CT * 128,), np.int8)
        col0 = 0
        for r, nb in enumerate(NB):
            cnt_k = int(np.searchsorted(-c["sorted_deg"], -r, side="left"))
            s = np.arange(cnt_k)
            g = nlo + c["order"][s]
            tr = t_s[node_start[g] + r]
            pair[col0 * 128 + s] = tr >> lg
            par[col0 * 128 + s] = (tr & (pfac - 1)).astype(np.int8)
            col0 += nb
        # int16 dma_gather layout: per round section, slots wrapped into 16
        # partitions ([16, 8*nb], slot i at [i%16, i//16]); the x8 partition
        # replication the gather engine wants is done on-device.
        idx16 = np.empty((16, 8 * CT), np.int16)
        col0 = 0
        for r, nb in enumerate(NB):
            vals = pair[col0 * 128:(col0 + nb) * 128]
            sec = vals.reshape(8 * nb, 16).T.astype(np.int16)     # [16, 8nb]
            idx16[:, 8 * col0:8 * (col0 + nb)] = sec
            col0 += nb
        # remainder mask [128, CT]: slot j*128+p -> [p, col0+j], bit-packed
        # along cols: packed[:, j] bit lg*k+b = bit b of rem at col j*per+k
        mask = par.reshape(CT, 128).T.astype(np.uint8)            # [128, CT]
        per = 8 // lg
        CP = -(-CT // per)
        padm = np.zeros((128, CP * per), np.uint8)
        padm[:, :CT] = mask
        packed = np.zeros((128, CP), np.uint8)
        for k in range(per):
            packed |= padm[:, k::per] << (lg * k)
        c["idx16"] = idx16
        c["mask"] = packed.view(np.int8)
    return cores, NB, NBLK


# --------------------------------------------------------------------------
# device kernel builder
# --------------------------------------------------------------------------

def _build_nc(cfg):
    TH = cfg["TH"]
    SH = TH // NCORES
    EMB_NB, EMB_NBLK = cfg["EMB_NB"], cfg["EMB_NBLK"]
    OFF_NB, OFF_NBLK = cfg["OFF_NB"], cfg["OFF_NBLK"]
    CE = max(1, sum(EMB_NB))
    CO = max(1, sum(OFF_NB))
    NCH = SH // CHUNK
    gcols = cfg.get("gcols", GCOLS)
    stage_bufs = cfg.get("stage_bufs", 2)
    nq = cfg.get("nqueues", 2)
    single_packet = cfg.get("single_packet", False)
    pfac_e = cfg.get("pfac_e", 2)
    pfac_o = cfg.get("pfac_o", 2)
    lg_e = pfac_e.bit_length() - 1
    lg_o = pfac_o.bit_length() - 1
    GROUP = [list(range(NCORES))]

    nc = bacc.Bacc(None, target_bir_lowering=False, debug=False,
                   num_devices=NCORES, num_swdge_queues=nq)

    # packed masks: emb 1 bit/slot (8 cols/byte), off 2 bits/slot (4/byte)
    CEP = -(-CE // 8)
    COP = -(-CO // 4)

    center_sh = nc.dram_tensor("center_sh", [SH, D], F16, kind="ExternalInput")
    offset_sh = nc.dram_tensor("offset_sh", [SH, D], I8, kind="ExternalInput")
    w1t = nc.dram_tensor("w1t", [D, D], F32, kind="ExternalInput")
    w2t = nc.dram_tensor("w2t", [D, D], F32, kind="ExternalInput")
    b1 = nc.dram_tensor("b1", [D, 1], F32, kind="ExternalInput")
    b2 = nc.dram_tensor("b2", [D, 1], F32, kind="ExternalInput")
    idx_e = nc.dram_tensor("idx_e", [16, 8 * CE], I16, kind="ExternalInput")
    idx_o = nc.dram_tensor("idx_o", [16, 8 * CO], I16, kind="ExternalInput")
    mask_e = nc.dram_tensor("mask_e", [128, CEP], I8, kind="ExternalInput")
    mask_o = nc.dram_tensor("mask_o", [128, COP], I8, kind="ExternalInput")

    tp_b = nc.dram_tensor("tp_b", [SH, 2 * D], F16)     # local table shard
    tp = nc.dram_tensor("tp", [TH, 2 * D], F16)         # AllGather output
    off_b = nc.dram_tensor("off_b", [SH, D], I8)
    offf = nc.dram_tensor("offf", [TH, D], I8)

    # single packed output: emb int8 | off int8 | emb per-row scales (f16
    # bitcast to byte pairs)
    OUTW = EMB_NBLK * D + OFF_NBLK * D + 2 * EMB_NBLK
    out_t = nc.dram_tensor("out_t", [128, OUTW], I8, kind="ExternalOutput")

    tp_grp = tp[:].rearrange("(u f) c -> u (f c)", f=pfac_e)    # [TH/fe, fe*2D]
    off_grp = offf[:].rearrange("(u f) c -> u (f c)", f=pfac_o)  # [TH/fo, fo*D]

    with tile.TileContext(nc) as tc:
        with (
            tc.tile_pool(name="persist", bufs=1) as pp,
            tc.tile_pool(name="ph0", bufs=3) as p0,
            tc.tile_pool(name="ph0psum", bufs=2, space="PSUM") as pps,
            tc.tile_pool(name="stage", bufs=stage_bufs) as ps,
            tc.tile_pool(name="selp", bufs=2) as psel,
        ):
            # ---- offset table: bounce shard -> AllGather (early) -----------
            nc.sync.dma_start(out=off_b[:], in_=offset_sh[:])
            nc.gpsimd.collective_compute(
                "AllGather", mybir.AluOpType.bypass, replica_groups=GROUP,
                ins=[off_b.ap().opt()], outs=[offf.ap().opt()])

            # ---- constants -------------------------------------------------
            w1t_sb = pp.tile([D, D], F32, tag="w1t")
            w2t_sb = pp.tile([D, D], F32, tag="w2t")
            b1_sb = pp.tile([D, 1], F32, tag="b1")
            b2_sb = pp.tile([D, 1], F32, tag="b2")
            ident = pp.tile([128, 128], F32, tag="ident")
            zrow = pp.tile([pfac_e, 2 * D], F16, tag="zrow")
            nc.sync.dma_start(out=w1t_sb[:], in_=w1t[:])
            nc.sync.dma_start(out=w2t_sb[:], in_=w2t[:])
            nc.sync.dma_start(out=b1_sb[:], in_=b1[:])
            nc.sync.dma_start(out=b2_sb[:], in_=b2[:])
            make_identity(nc, ident[:])
            nc.vector.memset(zrow[:], 0.0)

            # ---- persistent phase-1 state ---------------------------------
            # idx buffers are shipped [16, 8*C] and replicated to the
            # [128, 8*C] layout dma_gather wants (8 copies along partitions).
            idx_e_sb = pp.tile([128, 8 * CE], I16, tag="idx_e")
            idx_o_sb = pp.tile([128, 8 * CO], I16, tag="idx_o")
            mask_e_sb = pp.tile([128, CEP], I8, tag="mask_e")
            mask_o_sb = pp.tile([128, COP], I8, tag="mask_o")
            acc_e = pp.tile([128, EMB_NBLK * 128], F32, tag="acc_e")
            acc_o = pp.tile([128, OFF_NBLK * D], I8, tag="acc_o")
            for r in range(8):
                nc.sync.dma_start(out=idx_e_sb[16 * r:16 * (r + 1), :],
                                  in_=idx_e[:])
                nc.sync.dma_start(out=idx_o_sb[16 * r:16 * (r + 1), :],
                                  in_=idx_o[:])
            nc.sync.dma_start(out=mask_e_sb[:], in_=mask_e[:])
            nc.sync.dma_start(out=mask_o_sb[:], in_=mask_o[:])
            nc.vector.memset(acc_e[:], 0.0)
            nc.vector.memset(acc_o[:], 0.0)

            # unpack bit-packed remainder masks into per-bit predicate planes
            # (device layout: plane b, slot col j*per+k <- packed[:, j] bit
            # lg*k+b; nonzero byte == predicate true)
            def bit_masks(packed_sb, CP, lg, tag):
                per = 8 // lg
                mb = []
                for b in range(lg):
                    t = pp.tile([128, CP * per], I8, tag=f"mb_{tag}{b}")
                    t3 = t[:].rearrange("p (j k) -> p j k", k=per)
                    for k in range(per):
                        nc.vector.tensor_scalar(
                            out=t3[:, :, k], in0=packed_sb[:],
                            scalar1=1 << (lg * k + b), scalar2=None,
                            op0=mybir.AluOpType.bitwise_and)
                    mb.append(t)
                return mb

            mb_e = bit_masks(mask_e_sb, CEP, lg_e, "e")
            mb_o = bit_masks(mask_o_sb, COP, lg_o, "o")

            # gather pfac rows per descriptor, select the true row with a
            # log2(pfac)-stage predicated cascade, fold into the accumulator
            no_gather = cfg.get("no_gather", False)
            no_select = cfg.get("no_select", False)

            def gather_path(NB_list, grp_ap, idx_sb, mb, base, lg, acc_fn,
                            tag, q0, nqs, dt):
                col0 = 0
                call = 0
                f = 1 << lg
                for r, nb in enumerate(NB_list):
                    for j0 in range(0, nb, gcols):
                        w = min(gcols, nb - j0)
                        cl, cr = col0 + j0, col0 + j0 + w
                        st = ps.tile([128, gcols * f * base], dt,
                                     tag=f"stag_{tag}")
                        cur = st[:, :w * f * base].rearrange(
                            "p (j c) -> p j c", c=f * base)
                        if not no_gather:
                            nc.gpsimd.dma_gather(
                                out_ap=cur, in_ap=grp_ap,
                                idxs_ap=idx_sb[:, 8 * cl:8 * cr],
                                num_idxs=128 * w, num_idxs_reg=128 * w,
                                elem_size=f * base,
                                single_packet=single_packet,
                                queue_num=q0 + (call % nqs))
                        elif call < stage_bufs:
                            nc.vector.memset(st[:], 0.0)
                        call += 1
                        if no_select:
                            continue
                        cur2 = None
                        for b in range(lg - 1, -1, -1):
                            half = (1 << b) * base
                            nt = psel.tile([128, gcols * half], dt,
                                           tag=f"sel_{tag}{b}")
                            cur2 = nt[:, :w * half]
                            nxt = cur2.rearrange("p (j c) -> p j c", c=half)
                            nc.scalar.copy(out=nxt, in_=cur[:, :, 0:half])
                            nc.vector.copy_predicated(
                                out=nxt,
                                mask=mb[b][:, cl:cr].to_broadcast(
                                    [128, w, half]),
                                data=cur[:, :, half:2 * half])
                            cur = nxt
                        acc_fn(cur2, j0, w)
                    col0 += nb

            # ---- offset path: gather fp16 offsets, select, max ------------
            # (emitted first: needs only the early AllGather, overlaps the
            # table build)
            def acc_off(sv, j0, w):
                nc.vector.tensor_tensor(
                    out=acc_o[:, j0 * D:(j0 + w) * D],
                    in0=acc_o[:, j0 * D:(j0 + w) * D],
                    in1=sv, op=mybir.AluOpType.max)

            gather_path(OFF_NB, off_grp, idx_o_sb, mb_o, D, lg_o, acc_off,
                        "o", nq // 2, nq - nq // 2, I8)

            # ---- phase 0: local shard of node table  tp[v] = [e*c | e] ----
            for ch in range(NCH):
                sl = slice(ch * CHUNK, (ch + 1) * CHUNK)
                ld16 = p0.tile([128, (CHUNK // 128) * D], F16, tag="ld16")
                nc.sync.dma_start(
                    out=ld16[:].rearrange("p (q d) -> p q d", d=D),
                    in_=center_sh[sl, :].rearrange("(q p) d -> p q d", p=128))
                ld32 = p0.tile([128, (CHUNK // 128) * D], F32, tag="ld32")
                nc.vector.tensor_copy(out=ld32[:], in_=ld16[:])
                ctp = pps.tile([D, CHUNK], F32, tag="ctp")
                for q in range(CHUNK // 128):
                    nc.tensor.transpose(out=ctp[:, q * 128:(q + 1) * 128],
                                        in_=ld32[:, q * D:(q + 1) * D],
                                        identity=ident[:])
                ct = p0.tile([D, CHUNK], F32, tag="ct")
                nc.scalar.copy(out=ct[:], in_=ctp[:])
                ph = pps.tile([D, CHUNK], F32, tag="ph")
                nc.tensor.matmul(out=ph[:], lhsT=w1t_sb[:], rhs=ct[:],
                                 start=True, stop=True)
                hT = p0.tile([D, CHUNK], F32, tag="hT")
                nc.scalar.activation(out=hT[:], in_=ph[:],
                                     func=mybir.ActivationFunctionType.Relu,
                                     bias=b1_sb[:])
                pl = pps.tile([D, CHUNK], F32, tag="pl")
                nc.tensor.matmul(out=pl[:], lhsT=w2t_sb[:], rhs=hT[:],
                                 start=True, stop=True)
                eT = p0.tile([D, CHUNK], F32, tag="eT")
                nc.scalar.activation(out=eT[:], in_=pl[:],
                                     func=mybir.ActivationFunctionType.Exp,
                                     bias=b2_sb[:])
                pT = p0.tile([D, CHUNK], F32, tag="pT")
                nc.vector.tensor_tensor(out=pT[:], in0=eT[:], in1=ct[:],
                                        op=mybir.AluOpType.mult)
                pt = pps.tile([128, CHUNK], F32, tag="pt")
                for q in range(CHUNK // 128):
                    nc.tensor.transpose(out=pt[:, q * 128:q * 128 + D],
                                        in_=pT[:, q * 128:(q + 1) * 128],
                                        identity=ident[:D, :D])
                    nc.tensor.transpose(out=pt[:, q * 128 + D:(q + 1) * 128],
                                        in_=eT[:, q * 128:(q + 1) * 128],
                                        identity=ident[:D, :D])
                ot = p0.tile([128, CHUNK], F16, tag="ot")
                half = CHUNK // 2
                nc.vector.tensor_copy(out=ot[:, :half], in_=pt[:, :half])
                nc.scalar.copy(out=ot[:, half:], in_=pt[:, half:])
                nc.sync.dma_start(
                    out=tp_b[sl, :].rearrange("(q p) c -> p q c", p=128),
                    in_=ot[:].rearrange("p (q c) -> p q c", c=128),
                )

            # ---- AllGather the table, zero the sentinel group -------------
            nc.gpsimd.collective_compute(
                "AllGather", mybir.AluOpType.bypass, replica_groups=GROUP,
                ins=[tp_b.ap().opt()], outs=[tp.ap().opt()])
            nc.sync.dma_start(out=tp[TH - pfac_e:TH, :], in_=zrow[:])

            # ---- phase 1: emb gathers, select, add ------------------------
            def acc_emb(sv, j0, w):
                nc.vector.tensor_add(
                    out=acc_e[:, j0 * 128:(j0 + w) * 128],
                    in0=acc_e[:, j0 * 128:(j0 + w) * 128],
                    in1=sv)

            gather_path(EMB_NB, tp_grp, idx_e_sb, mb_e, 2 * D, lg_e, acc_emb,
                        "e", 0, nq // 2, F16)

            # ---- finals: v = num/den, l2norm, write out fp16 --------------
            acc3 = acc_e[:].rearrange("p (b c) -> p b c", c=128)
            num = acc3[:, :, 0:D]
            den = acc3[:, :, D:2 * D]
            nc.vector.tensor_scalar_max(den, den, 1e-30)
            nc.vector.reciprocal(den, den)
            v = pp.tile([128, EMB_NBLK * D], F32, tag="vfin")
            v3 = v[:].rearrange("p (b c) -> p b c", c=D)
            nc.vector.tensor_tensor(out=v3, in0=num, in1=den,
                                    op=mybir.AluOpType.mult)
            ssq = pp.tile([128, EMB_NBLK], F32, tag="ssq")
            for b in range(EMB_NBLK):
                sqs = p0.tile([128, D], F32, tag="sqscratch")
                nc.scalar.activation(
                    out=sqs[:], in_=v[:, b * D:(b + 1) * D],
                    func=mybir.ActivationFunctionType.Square,
                    accum_out=ssq[:, b:b + 1])
            nc.vector.tensor_scalar_max(ssq[:], ssq[:], 1e-24)
            nc.scalar.sqrt(out=ssq[:], in_=ssq[:])
            nc.vector.reciprocal(ssq[:], ssq[:])
            for b in range(EMB_NBLK):
                nc.scalar.mul(out=v[:, b * D:(b + 1) * D],
                              in_=v[:, b * D:(b + 1) * D],
                              mul=ssq[:, b:b + 1])
            # int8 output with per-row scale: q = round(v * 127/rowmax),
            # host dequantises with the f16 rowmax shipped alongside
            rmax = pp.tile([128, EMB_NBLK], F32, tag="rmax")
            nc.vector.tensor_reduce(
                out=rmax[:], in_=v3, axis=mybir.AxisListType.X,
                op=mybir.AluOpType.max, apply_absolute_value=True)
            nc.vector.tensor_scalar_max(rmax[:], rmax[:], 1e-6)
            rmax16 = pp.tile([128, EMB_NBLK], F16, tag="rmax16")
            nc.vector.tensor_copy(out=rmax16[:], in_=rmax[:])
            rinv = pp.tile([128, EMB_NBLK], F32, tag="rinv")
            # divide by the f16-rounded scale the host will multiply with
            nc.vector.tensor_copy(out=rinv[:], in_=rmax16[:])
            nc.vector.reciprocal(rinv[:], rinv[:])
            nc.vector.tensor_scalar_mul(rinv[:], rinv[:], 127.0)
            for b in range(EMB_NBLK):
                nc.scalar.mul(out=v[:, b * D:(b + 1) * D],
                              in_=v[:, b * D:(b + 1) * D],
                              mul=rinv[:, b:b + 1])
            nc.vector.tensor_scalar(
                out=v[:], in0=v[:], scalar1=127.0, scalar2=-127.0,
                op0=mybir.AluOpType.min, op1=mybir.AluOpType.max)
            vo = pp.tile([128, EMB_NBLK * D], I8, tag="vfin8")
            nc.vector.tensor_copy(out=vo[:], in_=v[:])
            EW = EMB_NBLK * D
            OW = OFF_NBLK * D
            nc.sync.dma_start(out=out_t[:, :EW], in_=vo[:])
            nc.sync.dma_start(out=out_t[:, EW:EW + OW], in_=acc_o[:])
            nc.sync.dma_start(
                out=out_t[:, EW + OW:].bitcast(F16), in_=rmax16[:])

    nc.compile()
    return nc


# --------------------------------------------------------------------------
# top-level entry
# --------------------------------------------------------------------------

def _prepare(inputs, TH, pfac_e=2, pfac_o=2):
    sent_e = (TH - pfac_e) // pfac_e
    sent_o = (TH - pfac_o) // pfac_o
    h1 = np.asarray(inputs["head1"])
    t1 = np.asarray(inputs["tail1"])
    h2 = np.asarray(inputs["head2"])
    t2 = np.asarray(inputs["tail2"])

    m = h1 < NV
    emb_cores, EMB_NB, EMB_NBLK = _shard_and_rounds(
        h1[m], t1[m], NCORES, sent_e, pfac_e)

    m1 = (h1 < NV) & (t1 >= NV)
    m2 = h2 < NV
    ho = np.concatenate([h1[m1], h2[m2]])
    to = np.concatenate([t1[m1], t2[m2]])
    off_cores, OFF_NB, OFF_NBLK = _shard_and_rounds(
        ho, to, NCORES, sent_o, pfac_o)

    all_center = np.concatenate(
        [inputs["visit_center"], inputs["ccs_center"], inputs["icd_center"]],
        0).astype(np.float32)
    all_offset = np.concatenate(
        [inputs["visit_offset"], inputs["ccs_offset"], inputs["icd_offset"]],
        0).astype(np.float32)
    center_pad = np.zeros((TH, D), np.float16)
    center_pad[:len(all_center)] = all_center.astype(np.float16)
    # offsets feed a segment max (monotone), so int8 quantisation with a
    # global scale survives the max exactly; dequantised on the host
    M = max(float(np.abs(all_offset).max()), 1e-12)
    offset_pad = np.zeros((TH, D), np.int8)
    offset_pad[:len(all_offset)] = np.rint(
        all_offset * (127.0 / M)).astype(np.int8)
    return dict(emb_cores=emb_cores, EMB_NB=EMB_NB, EMB_NBLK=EMB_NBLK,
                off_cores=off_cores, OFF_NB=OFF_NB, OFF_NBLK=OFF_NBLK,
                center16=center_pad, offset8=offset_pad,
                off_scale=M / 127.0)


_nc_cache = {}


def kernel(**inputs):
    TH = -(-NN // CHUNK) * CHUNK          # 57344
    SH = TH // NCORES
    pfac_e = _KERNEL_OVERRIDES.get("pfac_e", 2)
    pfac_o = _KERNEL_OVERRIDES.get("pfac_o", 4)
    prep = _prepare(inputs, TH, pfac_e, pfac_o)

    cfg = dict(TH=TH,
               EMB_NB=list(prep["EMB_NB"]), EMB_NBLK=prep["EMB_NBLK"],
               OFF_NB=list(prep["OFF_NB"]), OFF_NBLK=prep["OFF_NBLK"],
               gcols=12, stage_bufs=5, pfac_e=pfac_e, pfac_o=pfac_o)
    cfg.update(_KERNEL_OVERRIDES)
    key = tuple(sorted((k, tuple(v) if isinstance(v, list) else v)
                       for k, v in cfg.items()))
    nc = _nc_cache.get(key)
    if nc is None:
        nc = _build_nc(cfg)
        _nc_cache[key] = nc

    common = dict(
        w1t=np.ascontiguousarray(np.asarray(inputs["att_w1"]).T),
        w2t=np.ascontiguousarray(np.asarray(inputs["att_w2"]).T),
        b1=np.asarray(inputs["att_b1"]).reshape(D, 1),
        b2=np.asarray(inputs["att_b2"]).reshape(D, 1),
    )
    in_maps = []
    for k in range(NCORES):
        m = dict(common)
        m["center_sh"] = prep["center16"][k * SH:(k + 1) * SH]
        m["offset_sh"] = prep["offset8"][k * SH:(k + 1) * SH]
        m["idx_e"] = prep["emb_cores"][k]["idx16"]
        m["idx_o"] = prep["off_cores"][k]["idx16"]
        m["mask_e"] = prep["emb_cores"][k]["mask"]
        m["mask_o"] = prep["off_cores"][k]["mask"]
        in_maps.append(m)

    res = run_bass_kernel_spmd(nc, in_maps, core_ids=list(range(NCORES)))
    _last_results["res"] = res
    _last_results["nc"] = nc
    _last_results["in_maps"] = in_maps

    ENB, ONB = prep["EMB_NBLK"], prep["OFF_NBLK"]
    EW, OW = ENB * D, ONB * D
    emb = np.zeros((NV, D), np.float32)
    off = np.zeros((NV, D), np.float32)
    for k in range(NCORES):
        ce = prep["emb_cores"][k]
        co = prep["off_cores"][k]
        ot = res.results[k]["out_t"]
        eo = ot[:, :EW].reshape(128, ENB, D).astype(np.float32)
        sc = np.ascontiguousarray(ot[:, EW + OW:]).view(np.float16)
        eo *= sc.astype(np.float32)[:, :, None] / 127.0
        oo = (ot[:, EW:EW + OW].reshape(128, ONB, D).astype(np.float32)
              * prep["off_scale"])
        eo = eo.transpose(1, 0, 2).reshape(-1, D)
        oo = oo.transpose(1, 0, 2).reshape(-1, D)
        emb[ce["nlo"] + ce["order"]] = eo[:ce["nhi"] - ce["nlo"]]
        off[co["nlo"] + co["order"]] = oo[:co["nhi"] - co["nlo"]]
    return emb, off


# revision 48
# speedup vs baseline: 9.0862x; 1.0708x over previous
"""Trainium2 Bass kernel for nn_BoxLM_1168231104949 (gnn_message_passing).

Contract: kernel(**inputs) takes the FULL unsharded inputs (as produced by
setup_inputs()) and returns the full output (visit_final_emb,
visit_final_offset), each [50000, 64] float32.

Math notes (validated against the reference in fp64/numpy):
  * lam == 1.0  =>  visit_final_emb == l2norm(center_net(all_center[tail1],
    head1, N_NODES)[:NV]); the graph-2 center_net contributes exactly 0.
  * logits are tiny (|l| < ~1) so the segment softmax is computed with a raw
    exp (no per-segment max subtraction): out = num/den with
    num = seg_sum(exp(l)*emb), den = seg_sum(exp(l)).
  * exp(l) depends only on the tail node, so it is precomputed per node into
    a table T[v] = [exp(l(v))*center(v) | exp(l(v))] (fp16, 128 ch) and the
    edge work reduces to row gathers + segment sums.
  * The five masked/clamped segment maxes for visit_final_offset collapse to
    one masked segment max over (graph1: tail>=NV) + (graph2: all) edges,
    clamped at 0 (the accumulator initialised to 0 provides the clamp, and
    relu commutes with max so raw offsets are gathered).

Distribution: edges are sorted by head on the host and sharded into 8
contiguous head ranges balanced by edge count - each core owns a disjoint
slice of output nodes.  Within a core, nodes are ordered by degree into
"slots"; round r gathers the r-th edge of every node with degree > r via one
bulk dma_gather (slot i -> partition i%128, block i//128 - exactly the
accumulator layout).  dma_gather indices are int16, so rows are fetched in
PAIRS (pair idx = tail//2 <= 28671) and the correct half is selected
on-chip with a host-provided parity mask.

Wall time through the axon tunnel is dominated by host<->device transfer
(~40 MB/s measured), so inputs are shipped minimally: the node tables are
SHARDED 1/8 per core in fp16 and reconstructed on-device with NeuronLink
AllGather (each core builds its slice of the exp table from its center
shard before the gather); dma_gather index buffers are shipped un-replicated
[16, 8*CT] and expanded to the required [128, 8*CT] layout on-device; the
outputs return as fp16.
"""

import numpy as np

import concourse.bacc as bacc
import concourse.bass as bass
import concourse.mybir as mybir
import concourse.tile as tile
from concourse.bass_utils import run_bass_kernel_spmd
from concourse.masks import make_identity

F32 = mybir.dt.float32
F16 = mybir.dt.float16
I16 = mybir.dt.int16
I8 = mybir.dt.int8

NV = 50000
NN = 57300
D = 64
NCORES = 8

CHUNK = 512        # table rows per phase-0 chunk
GCOLS = 25         # max 128-slot blocks per gather call

_last_results = {}
_KERNEL_OVERRIDES = {}     # experiment knobs (gcols/stage_bufs/nqueues/...)


# --------------------------------------------------------------------------
# host-side index preprocessing
# --------------------------------------------------------------------------

def _shard_and_rounds(heads, tails, ncores, sent_grp, pfac):
    """Sort edges by head, shard into contiguous node ranges balanced by edge
    count, order nodes by degree desc, emit per-round int16 group-index
    buffers (un-replicated dma_gather layout) + remainder masks.

    Rows are fetched in groups of pfac consecutive table rows per descriptor
    (idx = tail // pfac); mask holds tail % pfac for the on-chip select.

    Returns (cores, NB, NBLK).  cores[k]: nlo/nhi/order/idx16/mask.
    NB[r] = 128-slot blocks in round r (uniform across cores).
    """
    lg = pfac.bit_length() - 1
    deg = np.bincount(heads, minlength=NV)
    cum = np.cumsum(deg)
    total = int(cum[-1])
    bounds = [0]
    for k in range(1, ncores):
        bounds.append(int(np.searchsorted(cum, total * k / ncores)))
    bounds.append(NV)

    order_e = np.argsort(heads, kind="stable")
    t_s = tails[order_e]
    node_start = np.zeros(NV + 1, np.int64)
    node_start[1:] = cum

    cores = []
    for k in range(ncores):
        nlo, nhi = bounds[k], bounds[k + 1]
        ldeg = deg[nlo:nhi]
        order = np.argsort(-ldeg, kind="stable")
        cores.append(dict(nlo=nlo, nhi=nhi, order=order,
                          sorted_deg=ldeg[order]))
    R = max(int(c["sorted_deg"][0]) if len(c["sorted_deg"]) else 0
            for c in cores)
    NBLK = max(-(-(c["nhi"] - c["nlo"]) // 128) for c in cores)
    NB = []
    for r in range(R):
        cnt = max(int(np.searchsorted(-c["sorted_deg"], -r, side="left"))
                  for c in cores)
        NB.append(max(1, -(-cnt // 128)))
    CT = sum(NB)
    for c in cores:
        nlo = c["nlo"]
        # per-slot tail (sent = pfac*sent_grp for padding), slot-major/round
        pair = np.full((CT * 128,), sent_grp, np.int32)
        par = np.zeros((CT * 128,), np.int8)
        col0 = 0
        for r, nb in enumerate(NB):
            cnt_k = int(np.searchsorted(-c["sorted_deg"], -r, side="left"))
            s = np.arange(cnt_k)
            g = nlo + c["order"][s]
            tr = t_s[node_start[g] + r]
            pair[col0 * 128 + s] = tr >> lg
            par[col0 * 128 + s] = (tr & (pfac - 1)).astype(np.int8)
            col0 += nb
        # int16 dma_gather layout: per round section, slots wrapped into 16
        # partitions ([16, 8*nb], slot i at [i%16, i//16]); the x8 partition
        # replication the gather engine wants is done on-device.
        idx16 = np.empty((16, 8 * CT), np.int16)
        col0 = 0
        for r, nb in enumerate(NB):
            vals = pair[col0 * 128:(col0 + nb) * 128]
            sec = vals.reshape(8 * nb, 16).T.astype(np.int16)     # [16, 8nb]
            idx16[:, 8 * col0:8 * (col0 + nb)] = sec
            col0 += nb
        # remainder mask [128, CT]: slot j*128+p -> [p, col0+j], bit-packed
        # along cols: packed[:, j] bit lg*k+b = bit b of rem at col j*per+k
        mask = par.reshape(CT, 128).T.astype(np.uint8)            # [128, CT]
        per = 8 // lg
        CP = -(-CT // per)
        padm = np.zeros((128, CP * per), np.uint8)
        padm[:, :CT] = mask
        packed = np.zeros((128, CP), np.uint8)
        for k in range(per):
            packed |= padm[:, k::per] << (lg * k)
        c["idx16"] = idx16
        c["mask"] = packed.view(np.int8)
    return cores, NB, NBLK


# --------------------------------------------------------------------------
# device kernel builder
# --------------------------------------------------------------------------

def _blob_layout(SH, CE, CO, CEP, COP):
    """Byte layout of the single fused per-core input blob (f32 sections
    first for alignment, then f16/i16, then i8)."""
    sec = [
        ("w1t", D * D * 4), ("w2t", D * D * 4), ("b1", D * 4), ("b2", D * 4),
        ("center", SH * D * 2), ("idx_e", 16 * 8 * CE * 2),
        ("idx_o", 16 * 8 * CO * 2), ("offset", SH * D),
        ("mask_e", 128 * CEP), ("mask_o", 128 * COP),
    ]
    lay = {}
    a = 0
    for nm, nb in sec:
        lay[nm] = (a, nb)
        a += nb
    lay["total"] = a
    return lay

def _build_nc(cfg):
    TH = cfg["TH"]
    SH = TH // NCORES
    EMB_NB, EMB_NBLK = cfg["EMB_NB"], cfg["EMB_NBLK"]
    OFF_NB, OFF_NBLK = cfg["OFF_NB"], cfg["OFF_NBLK"]
    CE = max(1, sum(EMB_NB))
    CO = max(1, sum(OFF_NB))
    NCH = SH // CHUNK
    gcols = cfg.get("gcols", GCOLS)
    stage_bufs = cfg.get("stage_bufs", 2)
    nq = cfg.get("nqueues", 2)
    single_packet = cfg.get("single_packet", False)
    pfac_e = cfg.get("pfac_e", 2)
    pfac_o = cfg.get("pfac_o", 2)
    lg_e = pfac_e.bit_length() - 1
    lg_o = pfac_o.bit_length() - 1
    GROUP = [list(range(NCORES))]

    nc = bacc.Bacc(None, target_bir_lowering=False, debug=False,
                   num_devices=NCORES, num_swdge_queues=nq)

    # packed masks: emb 1 bit/slot (8 cols/byte), off 2 bits/slot (4/byte)
    CEP = -(-CE // 8)
    COP = -(-CO // 4)

    LAY = _blob_layout(SH, CE, CO, CEP, COP)
    blob = nc.dram_tensor("blob", [LAY["total"]], I8, kind="ExternalInput")

    def bsec(nm, dt2, c):
        a, n = LAY[nm]
        ap = blob[a:a + n]
        if dt2 is not I8:
            ap = ap.bitcast(dt2)
        return ap.rearrange("(n c) -> n c", c=c)

    tp_b = nc.dram_tensor("tp_b", [SH, 2 * D], F16)     # local table shard
    tp = nc.dram_tensor("tp", [TH, 2 * D], F16)         # AllGather output
    center_sh = nc.dram_tensor("center_sh", [SH, D], F16)
    off_b = nc.dram_tensor("off_b", [SH, D], I8)
    offf = nc.dram_tensor("offf", [TH, D], I8)

    # single packed output: emb int8 | off int8 | emb per-row scales (f16
    # bitcast to byte pairs)
    OUTW = EMB_NBLK * D + OFF_NBLK * D + 2 * EMB_NBLK
    out_t = nc.dram_tensor("out_t", [128, OUTW], I8, kind="ExternalOutput")

    tp_grp = tp[:].rearrange("(u f) c -> u (f c)", f=pfac_e)    # [TH/fe, fe*2D]
    off_grp = offf[:].rearrange("(u f) c -> u (f c)", f=pfac_o)  # [TH/fo, fo*D]

    with tile.TileContext(nc) as tc:
        with (
            tc.tile_pool(name="persist", bufs=1) as pp,
            tc.tile_pool(name="ph0", bufs=3) as p0,
            tc.tile_pool(name="ph0psum", bufs=2, space="PSUM") as pps,
            tc.tile_pool(name="stage", bufs=stage_bufs) as ps,
            tc.tile_pool(name="selp", bufs=2) as psel,
        ):
            # ---- offset table: bounce shard -> AllGather (early) -----------
            nc.sync.dma_start(out=off_b[:], in_=bsec("offset", I8, D))
            nc.gpsimd.collective_compute(
                "AllGather", mybir.AluOpType.bypass, replica_groups=GROUP,
                ins=[off_b.ap().opt()], outs=[offf.ap().opt()])

            # unpack the center shard from the blob (DRAM->DRAM, once)
            nc.sync.dma_start(out=center_sh[:].bitcast(I8),
                              in_=bsec("center", I8, 2 * D))

            # ---- constants -------------------------------------------------
            w1t_sb = pp.tile([D, D], F32, tag="w1t")
            w2t_sb = pp.tile([D, D], F32, tag="w2t")
            b1_sb = pp.tile([D, 1], F32, tag="b1")
            b2_sb = pp.tile([D, 1], F32, tag="b2")
            ident = pp.tile([128, 128], F32, tag="ident")
            zrow = pp.tile([pfac_e, 2 * D], F16, tag="zrow")
            nc.sync.dma_start(out=w1t_sb[:], in_=bsec("w1t", F32, D))
            nc.sync.dma_start(out=w2t_sb[:], in_=bsec("w2t", F32, D))
            nc.sync.dma_start(out=b1_sb[:], in_=bsec("b1", F32, 1))
            nc.sync.dma_start(out=b2_sb[:], in_=bsec("b2", F32, 1))
            make_identity(nc, ident[:])
            nc.vector.memset(zrow[:], 0.0)

            # ---- persistent phase-1 state ---------------------------------
            # idx buffers are shipped [16, 8*C] and replicated to the
            # [128, 8*C] layout dma_gather wants (8 copies along partitions).
            idx_e_sb = pp.tile([128, 8 * CE], I16, tag="idx_e")
            idx_o_sb = pp.tile([128, 8 * CO], I16, tag="idx_o")
            mask_e_sb = pp.tile([128, CEP], I8, tag="mask_e")
            mask_o_sb = pp.tile([128, COP], I8, tag="mask_o")
            acc_e = pp.tile([128, EMB_NBLK * 128], F32, tag="acc_e")
            acc_o = pp.tile([128, OFF_NBLK * D], I8, tag="acc_o")
            for r in range(8):
                nc.sync.dma_start(out=idx_e_sb[16 * r:16 * (r + 1), :],
                                  in_=bsec("idx_e", I16, 8 * CE))
                nc.sync.dma_start(out=idx_o_sb[16 * r:16 * (r + 1), :],
                                  in_=bsec("idx_o", I16, 8 * CO))
            nc.sync.dma_start(out=mask_e_sb[:], in_=bsec("mask_e", I8, CEP))
            nc.sync.dma_start(out=mask_o_sb[:], in_=bsec("mask_o", I8, COP))
            nc.vector.memset(acc_e[:], 0.0)
            nc.vector.memset(acc_o[:], 0.0)

            # unpack bit-packed remainder masks into per-bit predicate planes
            # (device layout: plane b, slot col j*per+k <- packed[:, j] bit
            # lg*k+b; nonzero byte == predicate true)
            def bit_masks(packed_sb, CP, lg, tag):
                per = 8 // lg
                mb = []
                for b in range(lg):
                    t = pp.tile([128, CP * per], I8, tag=f"mb_{tag}{b}")
                    t3 = t[:].rearrange("p (j k) -> p j k", k=per)
                    for k in range(per):
                        nc.vector.tensor_scalar(
                            out=t3[:, :, k], in0=packed_sb[:],
                            scalar1=1 << (lg * k + b), scalar2=None,
                            op0=mybir.AluOpType.bitwise_and)
                    mb.append(t)
                return mb

            mb_e = bit_masks(mask_e_sb, CEP, lg_e, "e")
            mb_o = bit_masks(mask_o_sb, COP, lg_o, "o")

            # gather pfac rows per descriptor, select the true row with a
            # log2(pfac)-stage predicated cascade, fold into the accumulator
            no_gather = cfg.get("no_gather", False)
            no_select = cfg.get("no_select", False)

            def gather_path(NB_list, grp_ap, idx_sb, mb, base, lg, acc_fn,
                            tag, q0, nqs, dt):
                col0 = 0
                call = 0
                f = 1 << lg
                for r, nb in enumerate(NB_list):
                    for j0 in range(0, nb, gcols):
                        w = min(gcols, nb - j0)
                        cl, cr = col0 + j0, col0 + j0 + w
                        st = ps.tile([128, gcols * f * base], dt,
                                     tag=f"stag_{tag}")
                        cur = st[:, :w * f * base].rearrange(
                            "p (j c) -> p j c", c=f * base)
                        if not no_gather:
                            nc.gpsimd.dma_gather(
                                out_ap=cur, in_ap=grp_ap,
                                idxs_ap=idx_sb[:, 8 * cl:8 * cr],
                                num_idxs=128 * w, num_idxs_reg=128 * w,
                                elem_size=f * base,
                                single_packet=single_packet,
                                queue_num=q0 + (call % nqs))
                        elif call < stage_bufs:
                            nc.vector.memset(st[:], 0.0)
                        call += 1
                        if no_select:
                            continue
                        cur2 = None
                        for b in range(lg - 1, -1, -1):
                            half = (1 << b) * base
                            nt = psel.tile([128, gcols * half], dt,
                                           tag=f"sel_{tag}{b}")
                            cur2 = nt[:, :w * half]
                            nxt = cur2.rearrange("p (j c) -> p j c", c=half)
                            nc.scalar.copy(out=nxt, in_=cur[:, :, 0:half])
                            nc.vector.copy_predicated(
                                out=nxt,
                                mask=mb[b][:, cl:cr].to_broadcast(
                                    [128, w, half]),
                                data=cur[:, :, half:2 * half])
                            cur = nxt
                        acc_fn(cur2, j0, w)
                    col0 += nb

            # ---- offset path: gather fp16 offsets, select, max ------------
            # (emitted first: needs only the early AllGather, overlaps the
            # table build)
            def acc_off(sv, j0, w):
                nc.vector.tensor_tensor(
                    out=acc_o[:, j0 * D:(j0 + w) * D],
                    in0=acc_o[:, j0 * D:(j0 + w) * D],
                    in1=sv, op=mybir.AluOpType.max)

            gather_path(OFF_NB, off_grp, idx_o_sb, mb_o, D, lg_o, acc_off,
                        "o", nq // 2, nq - nq // 2, I8)

            # ---- phase 0: local shard of node table  tp[v] = [e*c | e] ----
            for ch in range(NCH):
                sl = slice(ch * CHUNK, (ch + 1) * CHUNK)
                ld16 = p0.tile([128, (CHUNK // 128) * D], F16, tag="ld16")
                nc.sync.dma_start(
                    out=ld16[:].rearrange("p (q d) -> p q d", d=D),
                    in_=center_sh[sl, :].rearrange("(q p) d -> p q d", p=128))
                ld32 = p0.tile([128, (CHUNK // 128) * D], F32, tag="ld32")
                nc.vector.tensor_copy(out=ld32[:], in_=ld16[:])
                ctp = pps.tile([D, CHUNK], F32, tag="ctp")
                for q in range(CHUNK // 128):
                    nc.tensor.transpose(out=ctp[:, q * 128:(q + 1) * 128],
                                        in_=ld32[:, q * D:(q + 1) * D],
                                        identity=ident[:])
                ct = p0.tile([D, CHUNK], F32, tag="ct")
                nc.scalar.copy(out=ct[:], in_=ctp[:])
                ph = pps.tile([D, CHUNK], F32, tag="ph")
                nc.tensor.matmul(out=ph[:], lhsT=w1t_sb[:], rhs=ct[:],
                                 start=True, stop=True)
                hT = p0.tile([D, CHUNK], F32, tag="hT")
                nc.scalar.activation(out=hT[:], in_=ph[:],
                                     func=mybir.ActivationFunctionType.Relu,
                                     bias=b1_sb[:])
                pl = pps.tile([D, CHUNK], F32, tag="pl")
                nc.tensor.matmul(out=pl[:], lhsT=w2t_sb[:], rhs=hT[:],
                                 start=True, stop=True)
                eT = p0.tile([D, CHUNK], F32, tag="eT")
                nc.scalar.activation(out=eT[:], in_=pl[:],
                                     func=mybir.ActivationFunctionType.Exp,
                                     bias=b2_sb[:])
                pT = p0.tile([D, CHUNK], F32, tag="pT")
                nc.vector.tensor_tensor(out=pT[:], in0=eT[:], in1=ct[:],
                                        op=mybir.AluOpType.mult)
                pt = pps.tile([128, CHUNK], F32, tag="pt")
                for q in range(CHUNK // 128):
                    nc.tensor.transpose(out=pt[:, q * 128:q * 128 + D],
                                        in_=pT[:, q * 128:(q + 1) * 128],
                                        identity=ident[:D, :D])
                    nc.tensor.transpose(out=pt[:, q * 128 + D:(q + 1) * 128],
                                        in_=eT[:, q * 128:(q + 1) * 128],
                                        identity=ident[:D, :D])
                ot = p0.tile([128, CHUNK], F16, tag="ot")
                half = CHUNK // 2
                nc.vector.tensor_copy(out=ot[:, :half], in_=pt[:, :half])
                nc.scalar.copy(out=ot[:, half:], in_=pt[:, half:])
                nc.sync.dma_start(
                    out=tp_b[sl, :].rearrange("(q p) c -> p q c", p=128),
                    in_=ot[:].rearrange("p (q c) -> p q c", c=128),
                )

            # ---- AllGather the table, zero the sentinel group -------------
            nc.gpsimd.collective_compute(
                "AllGather", mybir.AluOpType.bypass, replica_groups=GROUP,
                ins=[tp_b.ap().opt()], outs=[tp.ap().opt()])
            nc.sync.dma_start(out=tp[TH - pfac_e:TH, :], in_=zrow[:])

            # ---- phase 1: emb gathers, select, add ------------------------
            def acc_emb(sv, j0, w):
                nc.vector.tensor_add(
                    out=acc_e[:, j0 * 128:(j0 + w) * 128],
                    in0=acc_e[:, j0 * 128:(j0 + w) * 128],
                    in1=sv)

            gather_path(EMB_NB, tp_grp, idx_e_sb, mb_e, 2 * D, lg_e, acc_emb,
                        "e", 0, nq // 2, F16)

            # ---- finals: v = num/den, l2norm, write out fp16 --------------
            acc3 = acc_e[:].rearrange("p (b c) -> p b c", c=128)
            num = acc3[:, :, 0:D]
            den = acc3[:, :, D:2 * D]
            nc.vector.tensor_scalar_max(den, den, 1e-30)
            nc.vector.reciprocal(den, den)
            v = pp.tile([128, EMB_NBLK * D], F32, tag="vfin")
            v3 = v[:].rearrange("p (b c) -> p b c", c=D)
            nc.vector.tensor_tensor(out=v3, in0=num, in1=den,
                                    op=mybir.AluOpType.mult)
            ssq = pp.tile([128, EMB_NBLK], F32, tag="ssq")
            for b in range(EMB_NBLK):
                sqs = p0.tile([128, D], F32, tag="sqscratch")
                nc.scalar.activation(
                    out=sqs[:], in_=v[:, b * D:(b + 1) * D],
                    func=mybir.ActivationFunctionType.Square,
                    accum_out=ssq[:, b:b + 1])
            nc.vector.tensor_scalar_max(ssq[:], ssq[:], 1e-24)
            nc.scalar.sqrt(out=ssq[:], in_=ssq[:])
            nc.vector.reciprocal(ssq[:], ssq[:])
            for b in range(EMB_NBLK):
                nc.scalar.mul(out=v[:, b * D:(b + 1) * D],
                              in_=v[:, b * D:(b + 1) * D],
                              mul=ssq[:, b:b + 1])
            # int8 output with per-row scale: q = round(v * 127/rowmax),
            # host dequantises with the f16 rowmax shipped alongside
            rmax = pp.tile([128, EMB_NBLK], F32, tag="rmax")
            nc.vector.tensor_reduce(
                out=rmax[:], in_=v3, axis=mybir.AxisListType.X,
                op=mybir.AluOpType.max, apply_absolute_value=True)
            nc.vector.tensor_scalar_max(rmax[:], rmax[:], 1e-6)
            rmax16 = pp.tile([128, EMB_NBLK], F16, tag="rmax16")
            nc.vector.tensor_copy(out=rmax16[:], in_=rmax[:])
            rinv = pp.tile([128, EMB_NBLK], F32, tag="rinv")
            # divide by the f16-rounded scale the host will multiply with
            nc.vector.tensor_copy(out=rinv[:], in_=rmax16[:])
            nc.vector.reciprocal(rinv[:], rinv[:])
            nc.vector.tensor_scalar_mul(rinv[:], rinv[:], 127.0)
            for b in range(EMB_NBLK):
                nc.scalar.mul(out=v[:, b * D:(b + 1) * D],
                              in_=v[:, b * D:(b + 1) * D],
                              mul=rinv[:, b:b + 1])
            nc.vector.tensor_scalar(
                out=v[:], in0=v[:], scalar1=127.0, scalar2=-127.0,
                op0=mybir.AluOpType.min, op1=mybir.AluOpType.max)
            vo = pp.tile([128, EMB_NBLK * D], I8, tag="vfin8")
            nc.vector.tensor_copy(out=vo[:], in_=v[:])
            EW = EMB_NBLK * D
            OW = OFF_NBLK * D
            nc.sync.dma_start(out=out_t[:, :EW], in_=vo[:])
            nc.sync.dma_start(out=out_t[:, EW:EW + OW], in_=acc_o[:])
            nc.sync.dma_start(
                out=out_t[:, EW + OW:].bitcast(F16), in_=rmax16[:])

    nc.compile()
    return nc


# --------------------------------------------------------------------------
# top-level entry
# --------------------------------------------------------------------------

def _prepare(inputs, TH, pfac_e=2, pfac_o=2):
    sent_e = (TH - pfac_e) // pfac_e
    sent_o = (TH - pfac_o) // pfac_o
    h1 = np.asarray(inputs["head1"])
    t1 = np.asarray(inputs["tail1"])
    h2 = np.asarray(inputs["head2"])
    t2 = np.asarray(inputs["tail2"])

    m = h1 < NV
    emb_cores, EMB_NB, EMB_NBLK = _shard_and_rounds(
        h1[m], t1[m], NCORES, sent_e, pfac_e)

    m1 = (h1 < NV) & (t1 >= NV)
    m2 = h2 < NV
    ho = np.concatenate([h1[m1], h2[m2]])
    to = np.concatenate([t1[m1], t2[m2]])
    off_cores, OFF_NB, OFF_NBLK = _shard_and_rounds(
        ho, to, NCORES, sent_o, pfac_o)

    all_center = np.concatenate(
        [inputs["visit_center"], inputs["ccs_center"], inputs["icd_center"]],
        0).astype(np.float32)
    all_offset = np.concatenate(
        [inputs["visit_offset"], inputs["ccs_offset"], inputs["icd_offset"]],
        0).astype(np.float32)
    center_pad = np.zeros((TH, D), np.float16)
    center_pad[:len(all_center)] = all_center.astype(np.float16)
    # offsets feed a segment max (monotone), so int8 quantisation with a
    # global scale survives the max exactly; dequantised on the host
    M = max(float(np.abs(all_offset).max()), 1e-12)
    offset_pad = np.zeros((TH, D), np.int8)
    offset_pad[:len(all_offset)] = np.rint(
        all_offset * (127.0 / M)).astype(np.int8)
    return dict(emb_cores=emb_cores, EMB_NB=EMB_NB, EMB_NBLK=EMB_NBLK,
                off_cores=off_cores, OFF_NB=OFF_NB, OFF_NBLK=OFF_NBLK,
                center16=center_pad, offset8=offset_pad,
                off_scale=M / 127.0)


_nc_cache = {}


def kernel(**inputs):
    TH = -(-NN // CHUNK) * CHUNK          # 57344
    SH = TH // NCORES
    pfac_e = _KERNEL_OVERRIDES.get("pfac_e", 2)
    pfac_o = _KERNEL_OVERRIDES.get("pfac_o", 4)
    prep = _prepare(inputs, TH, pfac_e, pfac_o)

    cfg = dict(TH=TH,
               EMB_NB=list(prep["EMB_NB"]), EMB_NBLK=prep["EMB_NBLK"],
               OFF_NB=list(prep["OFF_NB"]), OFF_NBLK=prep["OFF_NBLK"],
               gcols=12, stage_bufs=5, pfac_e=pfac_e, pfac_o=pfac_o)
    cfg.update(_KERNEL_OVERRIDES)
    key = tuple(sorted((k, tuple(v) if isinstance(v, list) else v)
                       for k, v in cfg.items()))
    nc = _nc_cache.get(key)
    if nc is None:
        nc = _build_nc(cfg)
        _nc_cache[key] = nc

    w1t = np.ascontiguousarray(np.asarray(inputs["att_w1"]).T,
                               dtype=np.float32)
    w2t = np.ascontiguousarray(np.asarray(inputs["att_w2"]).T,
                               dtype=np.float32)
    b1 = np.asarray(inputs["att_b1"], dtype=np.float32).reshape(D, 1)
    b2 = np.asarray(inputs["att_b2"], dtype=np.float32).reshape(D, 1)

    def as_bytes(a):
        return np.ascontiguousarray(a).view(np.int8).ravel()

    in_maps = []
    for k in range(NCORES):
        parts = [
            as_bytes(w1t), as_bytes(w2t), as_bytes(b1), as_bytes(b2),
            as_bytes(prep["center16"][k * SH:(k + 1) * SH]),
            as_bytes(prep["emb_cores"][k]["idx16"]),
            as_bytes(prep["off_cores"][k]["idx16"]),
            as_bytes(prep["offset8"][k * SH:(k + 1) * SH]),
            as_bytes(prep["emb_cores"][k]["mask"]),
            as_bytes(prep["off_cores"][k]["mask"]),
        ]
        in_maps.append(dict(blob=np.concatenate(parts)))

    res = run_bass_kernel_spmd(nc, in_maps, core_ids=list(range(NCORES)))
    _last_results["res"] = res
    _last_results["nc"] = nc
    _last_results["in_maps"] = in_maps

    ENB, ONB = prep["EMB_NBLK"], prep["OFF_NBLK"]
    EW, OW = ENB * D, ONB * D
    emb = np.zeros((NV, D), np.float32)
    off = np.zeros((NV, D), np.float32)
    for k in range(NCORES):
        ce = prep["emb_cores"][k]
        co = prep["off_cores"][k]
        ot = res.results[k]["out_t"]
        eo = ot[:, :EW].reshape(128, ENB, D).astype(np.float32)
        sc = np.ascontiguousarray(ot[:, EW + OW:]).view(np.float16)
        eo *= sc.astype(np.float32)[:, :, None] / 127.0
        oo = (ot[:, EW:EW + OW].reshape(128, ONB, D).astype(np.float32)
              * prep["off_scale"])
        eo = eo.transpose(1, 0, 2).reshape(-1, D)
        oo = oo.transpose(1, 0, 2).reshape(-1, D)
        emb[ce["nlo"] + ce["order"]] = eo[:ce["nhi"] - ce["nlo"]]
        off[co["nlo"] + co["order"]] = oo[:co["nhi"] - co["nlo"]]
    return emb, off
